# revision 1
# baseline (speedup 1.0000x reference)
"""
Trainium2 Bass kernel for nn_MF_MGCN (5-band 2-layer GCN + MLP head).

Strategy (data-parallel over graphs, 8 NeuronCores):
  * Every graph has 19 nodes; edges never cross graphs.  Graph-local
    aggregation is done as dense block-diagonal matmuls with 6 graphs
    (114 rows) per block on the TensorEngine.
  * GCN layer 1 has a 1-channel input per band, so its [N,32] hidden is
    rank-1; only the scalar aggregate s = A_f_norm @ x is needed per band.
    With bt1 == 0, relu(BN1) @ W2 collapses exactly onto the two features
    (relu(z), relu(-z)), z = s - mean(s).  GCN layer 2 then aggregates
    just 2 channels per band.
  * BatchNorm uses global batch statistics, so the pipeline runs as four
    device launches with tiny partial-sum tensors combined on the host
    between launches (host does scalar/statistics math + layout only).
  * The structural edge pattern is identical for every graph (reference
    generator uses one random pattern + offsets), so layer-2 aggregation
    uses one shared block-diagonal stationary matrix and wide moving
    operands (few large matmuls).  If that (or bt1==0) ever fails to
    hold, a pure-numpy fallback reproduces the reference exactly.
"""

import sys

sys.path.insert(0, "/opt/trn_rl_repo")

import numpy as np
import ml_dtypes

BF16 = ml_dtypes.bfloat16

# Problem constants (hardcoded per task contract).
B = 32768
NN = 19
N = B * NN
BANDS = 5
EF, ES = 120, 60
EPS = 1e-5
NCORES = 8
G = B // NCORES           # graphs per core = 4096
SLOT = 6                  # graphs per 114-row block
NBLK = (G + SLOT - 1) // SLOT   # 683 blocks per core
NSLOT = NBLK * SLOT       # 4098 slots (2 zero-pad graphs)
P114 = SLOT * NN          # 114
CH1 = 64                  # L1 blocks per psum chunk (320 fp32 cols)
NCH1 = (NBLK + CH1 - 1) // CH1
CH2 = 51                  # L2 blocks per matmul     (510 fp32 cols)

_KERNEL_CACHE = {}


# --------------------------------------------------------------------------
# numpy fallback (exact reference math) -- used only if structural
# assumptions are violated; keeps kernel() correct for any inputs.
# --------------------------------------------------------------------------
def _bn_np(h, g, b):
    m = h.mean(0)
    v = h.var(0)
    return (h - m) / np.sqrt(v + EPS) * g + b


def _gcn_np(h, W, b, src, dst, ew, n):
    h = h @ W
    deg = np.zeros(n, np.float64)
    np.add.at(deg, dst, ew)
    deg += 1.0
    dinv = 1.0 / np.sqrt(deg)
    norm = dinv[src] * ew * dinv[dst]
    agg = np.zeros_like(h, dtype=np.float64)
    np.add.at(agg, dst, norm[:, None] * h[src])
    return agg + (dinv * dinv)[:, None] * h + b


def _fallback_numpy(i):
    x = np.asarray(i["x"], np.float64)
    sf, df = np.asarray(i["edge_index_func"][0]), np.asarray(i["edge_index_func"][1])
    ss, ds = np.asarray(i["edge_index_struct"][0]), np.asarray(i["edge_index_struct"][1])
    ew = np.asarray(i["edge_weight_func"], np.float64)
    ews = np.ones(ss.shape[0], np.float64)
    n = x.shape[0]
    outs = []
    for b in range(BANDS):
        h = _gcn_np(x[:, b : b + 1], np.asarray(i["W1"][b], np.float64),
                    np.asarray(i["b1"][b], np.float64), sf, df, ew, n)
        h = np.maximum(_bn_np(h, np.asarray(i["g1"][b], np.float64),
                              np.asarray(i["bt1"][b], np.float64)), 0)
        h = _gcn_np(h, np.asarray(i["W2"][b], np.float64),
                    np.asarray(i["b2"][b], np.float64), ss, ds, ews, n)
        h = np.maximum(_bn_np(h, np.asarray(i["g2"][b], np.float64),
                              np.asarray(i["bt2"][b], np.float64)), 0)
        outs.append(h.reshape(n // NN, NN * 2))
    xc = np.concatenate(outs, axis=1)
    h = np.maximum(_bn_np(xc @ np.asarray(i["lin1_W"], np.float64)
                          + np.asarray(i["lin1_b"], np.float64),
                          np.asarray(i["g3"], np.float64),
                          np.asarray(i["bt3"], np.float64)), 0)
    h = np.maximum(h @ np.asarray(i["lin2_W"], np.float64)
                   + np.asarray(i["lin2_b"], np.float64), 0)
    out = h @ np.asarray(i["lin3_W"], np.float64) + np.asarray(i["lin3_b"], np.float64)
    return out.astype(np.float32)


# --------------------------------------------------------------------------
# Bass kernel builders
# --------------------------------------------------------------------------
def _get_bass():
    import concourse.bacc as bacc
    import concourse.mybir as mybir
    from concourse import tile
    return bacc, mybir, tile


def _build_l1(hb):
    """Func aggregation, single launch: af streamed in double-buffered chunks."""
    bass, mybir, tile = _get_bass()
    f32, bf16 = mybir.dt.float32, mybir.dt.bfloat16
    nc = bass.Bacc(None, target_bir_lowering=False)
    xb = nc.dram_tensor("xb", [P114, hb, BANDS], bf16, kind="ExternalInput")
    af = nc.dram_tensor("af", [P114, hb, 128], bf16, kind="ExternalInput")
    s_out = nc.dram_tensor("s_out", [128, hb, BANDS], f32, kind="ExternalOutput")
    st1 = nc.dram_tensor("st1", [16, 1], f32, kind="ExternalOutput")
    with tile.TileContext(nc) as tc:
        with (
            tc.tile_pool(name="const", bufs=1) as cp,
            tc.tile_pool(name="wt", bufs=3) as wp,
            tc.tile_pool(name="ps", bufs=4, space="PSUM") as pp,
            tc.tile_pool(name="big", bufs=1) as bp,
        ):
            x_t = cp.tile([P114, hb, BANDS], bf16)
            nc.sync.dma_start(x_t[:], xb[:])
            s_t = bp.tile([128, hb, BANDS], f32)
            nch = (hb + CH1 - 1) // CH1
            for c in range(nch):
                nb = min(CH1, hb - c * CH1)
                a_t = wp.tile([P114, CH1, 128], bf16, tag="af")
                nc.sync.dma_start(a_t[:, :nb, :], af[:, c * CH1 : c * CH1 + nb, :])
                ps = pp.tile([128, CH1, BANDS], f32, tag="ps")
                for j in range(nb):
                    nc.tensor.matmul(
                        ps[:, j, :],
                        a_t[:, j, :],
                        x_t[:, c * CH1 + j, :],
                        start=True,
                        stop=True,
                    )
                nc.vector.tensor_copy(
                    out=s_t[:, c * CH1 : c * CH1 + nb, :], in_=ps[:, :nb, :]
                )
            sq_t = bp.tile([128, hb, BANDS], f32)
            nc.vector.tensor_tensor(out=sq_t[:], in0=s_t[:], in1=s_t[:],
                                    op=mybir.AluOpType.mult)
            part = cp.tile([128, 16], f32)
            nc.vector.memset(part[:], 0.0)
            for b in range(BANDS):
                nc.vector.tensor_reduce(out=part[:, b : b + 1], in_=s_t[:, :, b],
                                        axis=mybir.AxisListType.X,
                                        op=mybir.AluOpType.add)
                nc.vector.tensor_reduce(out=part[:, 5 + b : 6 + b], in_=sq_t[:, :, b],
                                        axis=mybir.AxisListType.X,
                                        op=mybir.AluOpType.add)
            ones = cp.tile([128, 1], f32)
            nc.vector.memset(ones[:], 1.0)
            pst = pp.tile([16, 1], f32, tag="st")
            nc.tensor.matmul(pst[:], part[:], ones[:], start=True, stop=True)
            st1_t = cp.tile([16, 1], f32)
            nc.vector.tensor_copy(out=st1_t[:], in_=pst[:])
            nc.sync.dma_start(st1[:], st1_t[:])
            nc.sync.dma_start(s_out[:], s_t[:])
    nc.compile()
    return nc


def _build_l2():
    """u=(relu(z),relu(-z)); v = As_blockdiag_shared @ u; stats(v)."""
    bass, mybir, tile = _get_bass()
    f32, bf16 = mybir.dt.float32, mybir.dt.bfloat16
    nc = bass.Bacc(None, target_bir_lowering=False)
    s_in = nc.dram_tensor("s_in", [128, NBLK, BANDS], f32, kind="ExternalInput")
    mub = nc.dram_tensor("mub", [128, 1, BANDS], f32, kind="ExternalInput")
    asb = nc.dram_tensor("asb", [P114, 128], bf16, kind="ExternalInput")
    v_out = nc.dram_tensor("v_out", [128, NBLK, 10], f32, kind="ExternalOutput")
    st2 = nc.dram_tensor("st2", [32, 1], f32, kind="ExternalOutput")
    nch = (NBLK + CH2 - 1) // CH2
    with tile.TileContext(nc) as tc:
        with (
            tc.tile_pool(name="const", bufs=1) as cp,
            tc.tile_pool(name="ps", bufs=4, space="PSUM") as pp,
            tc.tile_pool(name="big", bufs=1) as bp,
        ):
            s_t = bp.tile([128, NBLK, BANDS], f32)
            nc.sync.dma_start(s_t[:], s_in[:])
            mu_t = cp.tile([128, 1, BANDS], f32)
            nc.sync.dma_start(mu_t[:], mub[:])
            as_t = cp.tile([P114, 128], bf16)
            nc.sync.dma_start(as_t[:], asb[:])
            # z = s - mu  (mu broadcast over blocks)
            z_t = bp.tile([128, NBLK, BANDS], f32)
            nc.vector.tensor_tensor(
                out=z_t[:], in0=s_t[:],
                in1=mu_t[:].to_broadcast([128, NBLK, BANDS]),
                op=mybir.AluOpType.subtract,
            )
            # u columns: [u+ bands 0..4 | u- bands 0..4]
            u_t = bp.tile([P114, NBLK, 10], bf16)
            nc.scalar.activation(u_t[:, :, 0:5], z_t[:P114],
                                 mybir.ActivationFunctionType.Relu)
            nc.scalar.activation(u_t[:, :, 5:10], z_t[:P114],
                                 mybir.ActivationFunctionType.Relu, scale=-1.0)
            v_t = bp.tile([128, NBLK, 10], f32)
            for c in range(nch):
                nb = min(CH2, NBLK - c * CH2)
                ps = pp.tile([128, CH2, 10], f32, tag="ps")
                nc.tensor.matmul(
                    ps[:, :nb, :],
                    as_t[:],
                    u_t[:, c * CH2 : c * CH2 + nb, :],
                    start=True,
                    stop=True,
                )
                nc.vector.tensor_copy(
                    out=v_t[:, c * CH2 : c * CH2 + nb, :], in_=ps[:, :nb, :]
                )
            # stats: for each band: sum v+, v-, v+^2, v-^2, v+*v-
            sq_t = bp.tile([128, NBLK, 10], f32)
            nc.vector.tensor_tensor(out=sq_t[:], in0=v_t[:], in1=v_t[:],
                                    op=mybir.AluOpType.mult)
            p01_t = bp.tile([128, NBLK, BANDS], f32)
            nc.vector.tensor_tensor(out=p01_t[:], in0=v_t[:, :, 0:5],
                                    in1=v_t[:, :, 5:10], op=mybir.AluOpType.mult)
            part = cp.tile([128, 32], f32)
            nc.vector.memset(part[:], 0.0)
            for b in range(BANDS):
                for k, src in (
                    (0, v_t[:, :, b]), (5, v_t[:, :, 5 + b]),
                    (10, sq_t[:, :, b]), (15, sq_t[:, :, 5 + b]),
                    (20, p01_t[:, :, b]),
                ):
                    nc.vector.tensor_reduce(out=part[:, k + b : k + b + 1], in_=src,
                                            axis=mybir.AxisListType.X,
                                            op=mybir.AluOpType.add)
            ones = cp.tile([128, 1], f32)
            nc.vector.memset(ones[:], 1.0)
            pst = pp.tile([32, 1], f32, tag="st")
            nc.tensor.matmul(pst[:], part[:], ones[:], start=True, stop=True)
            st2_t = cp.tile([32, 1], f32)
            nc.vector.tensor_copy(out=st2_t[:], in_=pst[:])
            nc.sync.dma_start(st2[:], st2_t[:])
            nc.sync.dma_start(v_out[:], v_t[:])
    nc.compile()
    return nc


def _build_l3():
    """xc_k = relu(A_k*v0 + B_k*v1 + C_k); y1 = lin1(xc); stats(y1)."""
    bass, mybir, tile = _get_bass()
    f32 = mybir.dt.float32
    nc = bass.Bacc(None, target_bir_lowering=False)
    v0p = nc.dram_tensor("v0p", [95, G], f32, kind="ExternalInput")
    v1p = nc.dram_tensor("v1p", [95, G], f32, kind="ExternalInput")
    coef = nc.dram_tensor("coef", [95, 8], f32, kind="ExternalInput")
    w1k0 = nc.dram_tensor("w1k0", [95, 128], f32, kind="ExternalInput")
    w1k1 = nc.dram_tensor("w1k1", [95, 128], f32, kind="ExternalInput")
    l1b = nc.dram_tensor("l1b", [128, 1], f32, kind="ExternalInput")
    y1 = nc.dram_tensor("y1", [128, G], f32, kind="ExternalOutput")
    st3 = nc.dram_tensor("st3", [128, 2], f32, kind="ExternalOutput")
    with tile.TileContext(nc) as tc:
        with (
            tc.tile_pool(name="const", bufs=1) as cp,
            tc.tile_pool(name="ps", bufs=4, space="PSUM") as pp,
            tc.tile_pool(name="big", bufs=1) as bp,
        ):
            v0_t = bp.tile([95, G], f32)
            v1_t = bp.tile([95, G], f32)
            nc.sync.dma_start(v0_t[:], v0p[:])
            nc.sync.dma_start(v1_t[:], v1p[:])
            co_t = cp.tile([95, 8], f32)
            nc.sync.dma_start(co_t[:], coef[:])
            w0_t = cp.tile([95, 128], f32)
            w1_t = cp.tile([95, 128], f32)
            nc.sync.dma_start(w0_t[:], w1k0[:])
            nc.sync.dma_start(w1_t[:], w1k1[:])
            b_t = cp.tile([128, 1], f32)
            nc.sync.dma_start(b_t[:], l1b[:])
            xc = []
            for k in range(2):
                t0 = bp.tile([95, G], f32, tag=f"t0{k}")
                nc.vector.tensor_scalar(out=t0[:], in0=v0_t[:],
                                        scalar1=co_t[:, 3 * k : 3 * k + 1],
                                        scalar2=None, op0=mybir.AluOpType.mult)
                t1 = bp.tile([95, G], f32, tag=f"t1{k}")
                nc.vector.tensor_scalar(out=t1[:], in0=v1_t[:],
                                        scalar1=co_t[:, 3 * k + 1 : 3 * k + 2],
                                        scalar2=None, op0=mybir.AluOpType.mult)
                nc.vector.tensor_tensor(out=t0[:], in0=t0[:], in1=t1[:],
                                        op=mybir.AluOpType.add)
                nc.scalar.activation(t0[:], t0[:],
                                     mybir.ActivationFunctionType.Relu,
                                     bias=co_t[:, 3 * k + 2 : 3 * k + 3])
                xc.append(t0)
            y1_t = bp.tile([128, G], f32)
            for c in range(G // 512):
                sl = slice(c * 512, (c + 1) * 512)
                ps = pp.tile([128, 512], f32, tag="ps")
                nc.tensor.matmul(ps[:], w0_t[:], xc[0][:, sl], start=True, stop=False)
                nc.tensor.matmul(ps[:], w1_t[:], xc[1][:, sl], start=False, stop=True)
                nc.vector.tensor_scalar(out=y1_t[:, sl], in0=ps[:], scalar1=b_t[:, 0:1],
                                        scalar2=None, op0=mybir.AluOpType.add)
            sq_t = bp.tile([128, G], f32)
            nc.vector.tensor_tensor(out=sq_t[:], in0=y1_t[:], in1=y1_t[:],
                                    op=mybir.AluOpType.mult)
            st3_t = cp.tile([128, 2], f32)
            nc.vector.tensor_reduce(out=st3_t[:, 0:1], in_=y1_t[:],
                                    axis=mybir.AxisListType.X, op=mybir.AluOpType.add)
            nc.vector.tensor_reduce(out=st3_t[:, 1:2], in_=sq_t[:],
                                    axis=mybir.AxisListType.X, op=mybir.AluOpType.add)
            nc.sync.dma_start(st3[:], st3_t[:])
            nc.sync.dma_start(y1[:], y1_t[:])
    nc.compile()
    return nc


def _build_l4():
    """BN3+relu, lin2+relu, lin3."""
    bass, mybir, tile = _get_bass()
    f32 = mybir.dt.float32
    nc = bass.Bacc(None, target_bir_lowering=False)
    y1 = nc.dram_tensor("y1", [128, G], f32, kind="ExternalInput")
    g3b3 = nc.dram_tensor("g3b3", [128, 2], f32, kind="ExternalInput")
    w2 = nc.dram_tensor("w2", [128, 32], f32, kind="ExternalInput")
    l2b = nc.dram_tensor("l2b", [32, 1], f32, kind="ExternalInput")
    w3 = nc.dram_tensor("w3", [32, 2], f32, kind="ExternalInput")
    l3b = nc.dram_tensor("l3b", [2, 1], f32, kind="ExternalInput")
    yout = nc.dram_tensor("yout", [2, G], f32, kind="ExternalOutput")
    with tile.TileContext(nc) as tc:
        with (
            tc.tile_pool(name="const", bufs=1) as cp,
            tc.tile_pool(name="ps", bufs=4, space="PSUM") as pp,
            tc.tile_pool(name="big", bufs=1) as bp,
        ):
            y1_t = bp.tile([128, G], f32)
            nc.sync.dma_start(y1_t[:], y1[:])
            gb_t = cp.tile([128, 2], f32)
            nc.sync.dma_start(gb_t[:], g3b3[:])
            w2_t = cp.tile([128, 32], f32)
            nc.sync.dma_start(w2_t[:], w2[:])
            b2_t = cp.tile([32, 1], f32)
            nc.sync.dma_start(b2_t[:], l2b[:])
            w3_t = cp.tile([32, 2], f32)
            nc.sync.dma_start(w3_t[:], w3[:])
            b3_t = cp.tile([2, 1], f32)
            nc.sync.dma_start(b3_t[:], l3b[:])
            x2_t = bp.tile([128, G], f32)
            nc.vector.tensor_scalar(out=x2_t[:], in0=y1_t[:], scalar1=gb_t[:, 0:1],
                                    scalar2=None, op0=mybir.AluOpType.mult)
            nc.scalar.activation(x2_t[:], x2_t[:],
                                 mybir.ActivationFunctionType.Relu,
                                 bias=gb_t[:, 1:2])
            x3_t = bp.tile([32, G], f32)
            yo_t = bp.tile([2, G], f32)
            for c in range(G // 512):
                sl = slice(c * 512, (c + 1) * 512)
                ps2 = pp.tile([32, 512], f32, tag="ps2")
                nc.tensor.matmul(ps2[:], w2_t[:], x2_t[:, sl], start=True, stop=True)
                nc.scalar.activation(x3_t[:, sl], ps2[:],
                                     mybir.ActivationFunctionType.Relu,
                                     bias=b2_t[:, 0:1])
            for c in range(G // 512):
                sl = slice(c * 512, (c + 1) * 512)
                ps3 = pp.tile([2, 512], f32, tag="ps3")
                nc.tensor.matmul(ps3[:], w3_t[:], x3_t[:, sl], start=True, stop=True)
                nc.vector.tensor_scalar(out=yo_t[:, sl], in0=ps3[:],
                                        scalar1=b3_t[:, 0:1], scalar2=None,
                                        op0=mybir.AluOpType.add)
            nc.sync.dma_start(yout[:], yo_t[:])
    nc.compile()
    return nc


def _get_kernels():
    if "k" not in _KERNEL_CACHE:
        _KERNEL_CACHE["k"] = (_build_l1(NBLK), None,
                              _build_l2(), _build_l3(), _build_l4())
    return _KERNEL_CACHE["k"]


def _run(nc, in_maps, tag):
    from concourse.bass_utils import run_bass_kernel_spmd

    res = run_bass_kernel_spmd(nc, in_maps, core_ids=list(range(NCORES)))
    return res.results


# --------------------------------------------------------------------------
# main entry
# --------------------------------------------------------------------------
def kernel(**inputs) -> np.ndarray:
    x = np.asarray(inputs["x"], np.float32)
    eif = np.asarray(inputs["edge_index_func"])
    eis = np.asarray(inputs["edge_index_struct"])
    ew = np.asarray(inputs["edge_weight_func"], np.float32)
    W1 = np.asarray(inputs["W1"], np.float32)
    b1 = np.asarray(inputs["b1"], np.float32)
    g1 = np.asarray(inputs["g1"], np.float32)
    bt1 = np.asarray(inputs["bt1"], np.float32)
    W2 = np.asarray(inputs["W2"], np.float32)
    b2 = np.asarray(inputs["b2"], np.float32)
    g2 = np.asarray(inputs["g2"], np.float32)
    bt2 = np.asarray(inputs["bt2"], np.float32)
    lin1_W = np.asarray(inputs["lin1_W"], np.float32)
    lin1_b = np.asarray(inputs["lin1_b"], np.float32)
    g3 = np.asarray(inputs["g3"], np.float32)
    bt3 = np.asarray(inputs["bt3"], np.float32)
    lin2_W = np.asarray(inputs["lin2_W"], np.float32)
    lin2_b = np.asarray(inputs["lin2_b"], np.float32)
    lin3_W = np.asarray(inputs["lin3_W"], np.float32)
    lin3_b = np.asarray(inputs["lin3_b"], np.float32)

    sf, df = eif[0].astype(np.int64), eif[1].astype(np.int64)
    ss, ds = eis[0].astype(np.int64), eis[1].astype(np.int64)

    # --- structural-assumption checks (else exact numpy fallback) ---
    gs = ss // NN
    ok = np.array_equal(gs, ds // NN) and np.array_equal(
        gs, np.repeat(np.arange(B), ES)
    )
    gf = sf // NN
    ok = ok and np.array_equal(gf, df // NN) and np.array_equal(
        gf, np.repeat(np.arange(B), EF)
    )
    ssl, dsl = ss % NN, ds % NN
    ok = ok and np.array_equal(ssl.reshape(B, ES), np.broadcast_to(ssl[:ES], (B, ES)))
    ok = ok and np.array_equal(dsl.reshape(B, ES), np.broadcast_to(dsl[:ES], (B, ES)))
    ok = ok and np.abs(bt1).max() == 0.0
    if not ok:
        return _fallback_numpy(inputs)

    # --- host: build normalized func adjacency (transposed, self-loop folded)
    deg_f = np.bincount(df, weights=ew.astype(np.float64), minlength=N) + 1.0
    dinv_f = (1.0 / np.sqrt(deg_f)).astype(np.float32)
    norm_f = dinv_f[sf] * ew * dinv_f[df]
    sfl, dfl = sf % NN, df % NN
    idx = gf * (NN * NN) + sfl * NN + dfl
    AfT = np.bincount(idx, weights=norm_f.astype(np.float64),
                      minlength=B * NN * NN).astype(np.float32).reshape(B, NN, NN)
    dd = (dinv_f * dinv_f).reshape(B, NN)
    AfT[:, np.arange(NN), np.arange(NN)] += dd

    # --- host: shared structural adjacency (identical for all graphs)
    s0, d0 = ssl[:ES], dsl[:ES]
    deg_s = np.bincount(d0, minlength=NN).astype(np.float64) + 1.0
    dinv_s = 1.0 / np.sqrt(deg_s)
    AsT = np.zeros((NN, NN), np.float64)
    np.add.at(AsT, (s0, d0), dinv_s[s0] * dinv_s[d0])
    AsT[np.arange(NN), np.arange(NN)] += dinv_s * dinv_s
    asb = np.zeros((P114, 128), np.float32)
    for p in range(SLOT):
        asb[p * NN : (p + 1) * NN, p * NN : (p + 1) * NN] = AsT
    asb = asb.astype(BF16)

    # --- host: per-core packed inputs for L1
    x3 = x.reshape(B, NN, BANDS)
    l1_maps = []
    for c in range(NCORES):
        xs = np.zeros((NSLOT, NN, BANDS), np.float32)
        xs[:G] = x3[c * G : (c + 1) * G]
        xb = np.ascontiguousarray(
            xs.reshape(NBLK, SLOT, NN, BANDS).transpose(1, 2, 0, 3).reshape(
                P114, NBLK, BANDS)
        ).astype(BF16)
        Ac = np.zeros((NSLOT, NN, NN), np.float32)
        Ac[:G] = AfT[c * G : (c + 1) * G]
        Ac = Ac.reshape(NBLK, SLOT, NN, NN)
        Z = np.zeros((NBLK, P114, 128), np.float32)
        for p in range(SLOT):
            Z[:, p * NN : (p + 1) * NN, p * NN : (p + 1) * NN] = Ac[:, p]
        af = np.ascontiguousarray(Z.transpose(1, 0, 2)).astype(BF16)
        l1_maps.append({"xb": xb, "af": af})

    try:
        return _device_pipeline(l1_maps, asb, AsT.sum(0).astype(np.float64),
                                W1, g1, W2, b2, g2, bt2,
                                lin1_W, lin1_b, g3, bt3, lin2_W, lin2_b,
                                lin3_W, lin3_b)
    except Exception as e:
        import traceback
        print(f"device pipeline failed ({e}); numpy fallback", file=sys.stderr)
        traceback.print_exc()
        return _fallback_numpy(inputs)


def _device_pipeline(l1_maps, asb, cs, W1, g1, W2, b2, g2, bt2, lin1_W, lin1_b,
                     g3, bt3, lin2_W, lin2_b, lin3_W, lin3_b):
    ncs = _get_kernels()
    r1 = _run(ncs[0], l1_maps, "l1")

    # --- host: BN1 statistics + mu tile
    st = sum(r["st1"][:, 0].astype(np.float64) for r in r1)
    mu1 = (st[:BANDS] / N).astype(np.float32)
    var1 = (st[BANDS : 2 * BANDS] / N - mu1.astype(np.float64) ** 2).astype(np.float32)
    # h1 = s*W1row + b1 -> BN1 -> relu -> @W2 collapses to P*relu(z)+Q*relu(-z)
    w1r = W1[:, 0, :]                                # [BANDS, 32]
    rs1 = 1.0 / np.sqrt(var1[:, None] * w1r * w1r + EPS)   # [BANDS, 32]
    a = w1r * rs1 * g1                               # [BANDS, 32]
    Pk = np.einsum("bj,bjk->bk", np.maximum(a, 0), W2)     # [BANDS, 2]
    Qk = np.einsum("bj,bjk->bk", np.maximum(-a, 0), W2)    # [BANDS, 2]
    mub = np.broadcast_to(mu1[None, None, :], (128, 1, BANDS)).astype(np.float32)
    mub = np.ascontiguousarray(mub)

    l2_maps = [{"s_in": np.ascontiguousarray(r["s_out"].reshape(128, NBLK, BANDS)),
                "mub": mub, "asb": asb} for r in r1]
    r2 = _run(ncs[2], l2_maps, "l2")

    # --- host: BN2 statistics -> affine coefficients on (v+, v-)
    st2 = sum(r["st2"][:, 0].astype(np.float64) for r in r2)
    npad = NCORES * (NSLOT - G)            # pad graph slots across cores
    for b in range(BANDS):
        up_c = max(-float(mu1[b]), 0.0)
        um_c = max(float(mu1[b]), 0.0)
        svp, svm = up_c * cs, um_c * cs
        st2[0 + b] -= npad * svp.sum()
        st2[5 + b] -= npad * svm.sum()
        st2[10 + b] -= npad * (svp ** 2).sum()
        st2[15 + b] -= npad * (svm ** 2).sum()
        st2[20 + b] -= npad * (svp * svm).sum()
    mVp, mVm = st2[0:5] / N, st2[5:10] / N
    eVp2, eVm2, eVpm = st2[10:15] / N, st2[15:20] / N, st2[20:25] / N
    vVp = eVp2 - mVp**2
    vVm = eVm2 - mVm**2
    cVpm = eVpm - mVp * mVm
    # h2_k = Pk*v+ + Qk*v- + b2_k
    mu2 = Pk * mVp[:, None] + Qk * mVm[:, None] + b2          # [BANDS, 2]
    var2 = (Pk**2 * vVp[:, None] + Qk**2 * vVm[:, None]
            + 2 * Pk * Qk * cVpm[:, None])
    rs2 = 1.0 / np.sqrt(var2 + EPS)
    Ak = (Pk * rs2 * g2).astype(np.float32)                   # [BANDS, 2]
    Bk = (Qk * rs2 * g2).astype(np.float32)
    Ck = ((b2 - mu2) * rs2 * g2 + bt2).astype(np.float32)
    coef = np.zeros((95, 8), np.float32)
    for k in range(2):
        coef[:, 3 * k + 0] = np.repeat(Ak[:, k], NN)
        coef[:, 3 * k + 1] = np.repeat(Bk[:, k], NN)
        coef[:, 3 * k + 2] = np.repeat(Ck[:, k], NN)
    # lin1 row split by k-parity: row(band, n, k) = band*38 + n*2 + k
    ridx = (np.arange(BANDS)[:, None] * 2 * NN
            + np.arange(NN)[None, :] * 2).reshape(-1)         # [95]
    w1k0 = np.ascontiguousarray(lin1_W[ridx]).astype(np.float32)
    w1k1 = np.ascontiguousarray(lin1_W[ridx + 1]).astype(np.float32)
    l1bv = lin1_b.reshape(128, 1).astype(np.float32)

    l3_maps = []
    for c in range(NCORES):
        vo = r2[c]["v_out"].reshape(128, NBLK, 2, BANDS)[:P114]
        vo = vo.reshape(SLOT, NN, NBLK, 2, BANDS)
        # -> [band, n, pm, block, slot] -> graphs
        vp = vo.transpose(4, 1, 3, 2, 0).reshape(BANDS, NN, 2, NSLOT)[:, :, :, :G]
        v0p = np.ascontiguousarray(vp[:, :, 0, :].reshape(95, G))
        v1p = np.ascontiguousarray(vp[:, :, 1, :].reshape(95, G))
        l3_maps.append({"v0p": v0p, "v1p": v1p, "coef": coef,
                        "w1k0": w1k0, "w1k1": w1k1, "l1b": l1bv})
    r3 = _run(ncs[3], l3_maps, "l3")

    # --- host: BN3 statistics
    st3 = sum(r["st3"].astype(np.float64) for r in r3)
    mu3 = st3[:, 0] / B
    var3 = st3[:, 1] / B - mu3**2
    G3 = (g3 / np.sqrt(var3 + EPS)).astype(np.float32)
    B3 = (bt3 - mu3 * G3).astype(np.float32)
    g3b3 = np.ascontiguousarray(np.stack([G3, B3], axis=1))
    l4_maps = [{"y1": r["y1"], "g3b3": g3b3,
                "w2": np.ascontiguousarray(lin2_W),
                "l2b": lin2_b.reshape(32, 1),
                "w3": np.ascontiguousarray(lin3_W),
                "l3b": lin3_b.reshape(2, 1)} for r in r3]
    r4 = _run(ncs[4], l4_maps, "l4")

    out = np.empty((B, 2), np.float32)
    for c in range(NCORES):
        out[c * G : (c + 1) * G] = r4[c]["yout"].T
    return out



# revision 14
# speedup vs baseline: 23.8733x; 23.8733x over previous
"""
Trainium2 Bass kernel for nn_MF_MGCN (5-band 2-layer GCN + MLP head).

Single fused device launch (vs the 4-launch baseline):
  * BatchNorm statistics are reduced across the 8 cores with on-device
    AllReduce collectives (3x tiny: 10, 25, and 256 floats), and the BN
    coefficient algebra runs on-device, so no host round trips remain.
  * The func adjacency ships packed as [114, NBLK, 19] bf16 (~3MB/core
    instead of ~20MB/core block-diagonal); the block-diagonal stationary
    tiles are assembled in SBUF by strided DMAs over a zeroed background.
  * Math identical to the baseline derivation: GCN1 on a 1-channel input
    is rank-1, so relu(BN1) @ W2 collapses onto (relu(z), relu(-z)) and
    GCN2 aggregates just 2 channels/band through one shared structural
    block-diagonal matrix.
  * The executable (jit of shard_map over 8 cores) is cached across
    calls, so warm runs skip retrace/recompile.

All compute-engine operands keep partition base 0 (hardware restricts
bases to {0,32,64}); any partition reshuffling goes through DMA.

If structural assumptions fail (shared struct pattern, bt1 == 0), a pure
numpy fallback reproduces the reference exactly.
"""

import sys

sys.path.insert(0, "/opt/trn_rl_repo")

import numpy as np
import ml_dtypes

BF16 = ml_dtypes.bfloat16

# Problem constants (hardcoded per task contract).
B = 32768
NN = 19
N = B * NN
BANDS = 5
EF, ES = 120, 60
EPS = 1e-5
NCORES = 8
G = B // NCORES           # graphs per core = 4096
SLOT = 6                  # graphs per 114-row block
P114 = SLOT * NN          # 114

_KERNEL_CACHE = {}
_RUNNER_CACHE = {}


# --------------------------------------------------------------------------
# numpy fallback (exact reference math)
# --------------------------------------------------------------------------
def _bn_np(h, g, b):
    m = h.mean(0)
    v = h.var(0)
    return (h - m) / np.sqrt(v + EPS) * g + b


def _gcn_np(h, W, b, src, dst, ew, n):
    h = h @ W
    deg = np.zeros(n, np.float64)
    np.add.at(deg, dst, ew)
    deg += 1.0
    dinv = 1.0 / np.sqrt(deg)
    norm = dinv[src] * ew * dinv[dst]
    agg = np.zeros_like(h, dtype=np.float64)
    np.add.at(agg, dst, norm[:, None] * h[src])
    return agg + (dinv * dinv)[:, None] * h + b


def _fallback_numpy(i):
    x = np.asarray(i["x"], np.float64)
    sf, df = np.asarray(i["edge_index_func"][0]), np.asarray(i["edge_index_func"][1])
    ss, ds = np.asarray(i["edge_index_struct"][0]), np.asarray(i["edge_index_struct"][1])
    ew = np.asarray(i["edge_weight_func"], np.float64)
    ews = np.ones(ss.shape[0], np.float64)
    n = x.shape[0]
    outs = []
    for b in range(BANDS):
        h = _gcn_np(x[:, b : b + 1], np.asarray(i["W1"][b], np.float64),
                    np.asarray(i["b1"][b], np.float64), sf, df, ew, n)
        h = np.maximum(_bn_np(h, np.asarray(i["g1"][b], np.float64),
                              np.asarray(i["bt1"][b], np.float64)), 0)
        h = _gcn_np(h, np.asarray(i["W2"][b], np.float64),
                    np.asarray(i["b2"][b], np.float64), ss, ds, ews, n)
        h = np.maximum(_bn_np(h, np.asarray(i["g2"][b], np.float64),
                              np.asarray(i["bt2"][b], np.float64)), 0)
        outs.append(h.reshape(n // NN, NN * 2))
    xc = np.concatenate(outs, axis=1)
    h = np.maximum(_bn_np(xc @ np.asarray(i["lin1_W"], np.float64)
                          + np.asarray(i["lin1_b"], np.float64),
                          np.asarray(i["g3"], np.float64),
                          np.asarray(i["bt3"], np.float64)), 0)
    h = np.maximum(h @ np.asarray(i["lin2_W"], np.float64)
                   + np.asarray(i["lin2_b"], np.float64), 0)
    out = h @ np.asarray(i["lin3_W"], np.float64) + np.asarray(i["lin3_b"], np.float64)
    return out.astype(np.float32)


# --------------------------------------------------------------------------
# fused Bass kernel builder
# --------------------------------------------------------------------------
def _build_fused(nblk, g_per_core, ncores, n_total, b_total):
    import concourse.bacc as bacc
    import concourse.mybir as mybir
    from concourse import tile

    f32, bf16 = mybir.dt.float32, mybir.dt.bfloat16
    Relu = mybir.ActivationFunctionType.Relu
    Sqrt = mybir.ActivationFunctionType.Sqrt
    ALU = mybir.AluOpType
    AX = mybir.AxisListType

    nslot = nblk * SLOT
    npad = nslot - g_per_core          # pad graph slots (live in last block)
    assert 0 <= npad < SLOT
    CH1 = min(48, nblk)                # L1 blocks per chunk
    CH2 = min(51, nblk)                # L2 blocks per chunk (510 moving cols)
    rgroups = [list(range(ncores))]

    nc = bacc.Bacc(None, target_bir_lowering=False, num_devices=ncores)

    xb = nc.dram_tensor("xb", [P114, nblk, BANDS], bf16, kind="ExternalInput")
    afp = nc.dram_tensor("afp", [P114, nblk, NN], bf16, kind="ExternalInput")
    asb = nc.dram_tensor("asb", [P114, 128], bf16, kind="ExternalInput")
    w1r = nc.dram_tensor("w1r", [BANDS, 32], f32, kind="ExternalInput")
    g1w = nc.dram_tensor("g1w", [BANDS, 32], f32, kind="ExternalInput")
    w2k = nc.dram_tensor("w2k", [BANDS, 4, 32], f32, kind="ExternalInput")
    cvec = nc.dram_tensor("cvec", [BANDS, 6], f32, kind="ExternalInput")
    w1s = nc.dram_tensor("w1s", [NN, 10, 128], bf16, kind="ExternalInput")
    l1bv = nc.dram_tensor("l1bv", [128, 3], f32, kind="ExternalInput")
    w2l = nc.dram_tensor("w2l", [128, 32], bf16, kind="ExternalInput")
    l2b = nc.dram_tensor("l2b", [32, 1], f32, kind="ExternalInput")
    w3l = nc.dram_tensor("w3l", [32, 2], bf16, kind="ExternalInput")
    l3b = nc.dram_tensor("l3b", [2, 1], f32, kind="ExternalInput")
    yout = nc.dram_tensor("yout", [2, nslot], f32, kind="ExternalOutput")

    inv_n = 1.0 / float(n_total)
    inv_b = 1.0 / float(b_total)

    with tile.TileContext(nc) as tc:
        with (
            tc.tile_pool(name="const", bufs=1) as cp,
            tc.tile_pool(name="big", bufs=1) as bp,
            tc.tile_pool(name="scr", bufs=2) as sp,
            tc.tile_pool(name="scr1", bufs=1) as sp1,
            tc.tile_pool(name="dram", bufs=1, space="DRAM") as dp,
        ):
            # ---------- persistent loads ----------
            x_t = bp.tile([P114, nblk, BANDS], bf16)
            nc.sync.dma_start(x_t[:], xb[:])
            as_t = cp.tile([P114, 128], bf16)
            nc.sync.dma_start(as_t[:], asb[:])
            w1r_t = cp.tile([BANDS, 32], f32)
            nc.sync.dma_start(w1r_t[:], w1r[:])
            g1w_t = cp.tile([BANDS, 32], f32)
            nc.sync.dma_start(g1w_t[:], g1w[:])
            w2k_t = cp.tile([BANDS, 4, 32], f32)
            nc.sync.dma_start(w2k_t[:], w2k[:])
            cv_t = cp.tile([BANDS, 6], f32)
            nc.sync.dma_start(cv_t[:], cvec[:])
            w1s_t = cp.tile([NN, 10, 128], bf16)
            nc.sync.dma_start(w1s_t[:], w1s[:])
            l1b_t = cp.tile([128, 3], f32)
            nc.sync.dma_start(l1b_t[:], l1bv[:])
            w2l_t = cp.tile([128, 32], bf16)
            nc.sync.dma_start(w2l_t[:], w2l[:])
            l2b_t = cp.tile([32, 1], f32)
            nc.sync.dma_start(l2b_t[:], l2b[:])
            w3l_t = cp.tile([32, 2], bf16)
            nc.sync.dma_start(w3l_t[:], w3l[:])
            l3b_t = cp.tile([2, 1], f32)
            nc.sync.dma_start(l3b_t[:], l3b[:])
            ones_t = cp.tile([128, 1], f32)
            nc.vector.memset(ones_t[:], 1.0)
            eps128 = cp.tile([128, 1], f32)
            nc.vector.memset(eps128[:], EPS)
            zpad = cp.tile([P114 - NN, 2 * BANDS], bf16)
            nc.vector.memset(zpad[:], 0.0)

            # DRAM bounce buffers for collectives
            cc1_in = dp.tile([10, 1], f32)
            cc1_out = dp.tile([1, 10], f32)
            cc2_in = dp.tile([25, 1], f32)
            cc2_out = dp.tile([1, 25], f32)
            cc3_in = dp.tile([128, 2], f32)
            cc3_out = dp.tile([128, 2], f32)
            abc_d = dp.tile([1, 30], f32)

            # ---------- L1: s = AfT_blockdiag @ x ----------
            s_t = bp.tile([128, nblk, BANDS], f32)
            a0 = bp.tile([P114, CH1, 128], bf16)
            a1 = bp.tile([P114, CH1, 128], bf16)
            nc.vector.memset(a0[:], 0.0)
            nc.vector.memset(a1[:], 0.0)
            a_bufs = [a0, a1]
            nch1 = (nblk + CH1 - 1) // CH1
            with tc.tile_pool(name="ps1", bufs=4, space="PSUM") as pp1:
                for c in range(nch1):
                    c0 = c * CH1
                    nb = min(CH1, nblk - c0)
                    a_t = a_bufs[c % 2]
                    for p in range(SLOT):
                        nc.sync.dma_start(
                            a_t[p * NN : (p + 1) * NN, :nb, p * NN : (p + 1) * NN],
                            afp[p * NN : (p + 1) * NN, c0 : c0 + nb, :],
                        )
                    ps = pp1.tile([128, CH1, BANDS], f32, tag="ps1")
                    for j in range(nb):
                        nc.tensor.matmul(
                            ps[:, j, :], a_t[:, j, :], x_t[:, c0 + j, :],
                            start=True, stop=True,
                        )
                    nc.vector.tensor_copy(out=s_t[:, c0 : c0 + nb, :],
                                          in_=ps[:, :nb, :])

                # ---------- L1 stats: per-band sum(s), sum(s^2) ----------
                part1 = cp.tile([128, 10], f32)
                for b in range(BANDS):
                    nc.vector.tensor_reduce(out=part1[:, b : b + 1],
                                            in_=s_t[:, :, b], axis=AX.X, op=ALU.add)
                    scr = sp.tile([128, nblk], f32, tag="scr")
                    nc.vector.tensor_tensor(out=scr[:], in0=s_t[:, :, b],
                                            in1=s_t[:, :, b], op=ALU.mult)
                    nc.vector.tensor_reduce(out=part1[:, 5 + b : 6 + b],
                                            in_=scr[:], axis=AX.X, op=ALU.add)
                pst1 = pp1.tile([10, 1], f32, tag="pst1")
                nc.tensor.matmul(pst1[:], part1[:], ones_t[:], start=True, stop=True)
                st1_t = cp.tile([10, 1], f32)
                nc.vector.tensor_copy(out=st1_t[:], in_=pst1[:])

            nc.sync.dma_start(cc1_in[:], st1_t[:])
            nc.gpsimd.collective_compute(
                "AllReduce", ALU.add, replica_groups=rgroups,
                ins=[cc1_in[:].opt()], outs=[cc1_out[:].opt()],
            )

            # ---------- BN1 coefficients ----------
            mv_t = cp.tile([BANDS, 2], f32)          # col0 = mu1, col1 = E[s^2]
            nc.sync.dma_start(mv_t[:, 0:1],
                              cc1_out[:, 0:BANDS].rearrange("one b -> b one"))
            nc.sync.dma_start(mv_t[:, 1:2],
                              cc1_out[:, BANDS : 2 * BANDS]
                              .rearrange("one b -> b one"))
            nc.vector.tensor_scalar(out=mv_t[:], in0=mv_t[:], scalar1=inv_n,
                                    scalar2=None, op0=ALU.mult)
            var5 = cp.tile([BANDS, 1], f32)
            nc.vector.tensor_tensor(out=var5[:], in0=mv_t[:, 0:1], in1=mv_t[:, 0:1],
                                    op=ALU.mult)
            nc.vector.tensor_tensor(out=var5[:], in0=mv_t[:, 1:2], in1=var5[:],
                                    op=ALU.subtract)
            # a = w1r * rsqrt(var*w1r^2 + eps) * g1   [5, 32]
            a5 = cp.tile([BANDS, 32], f32)
            nc.vector.tensor_tensor(out=a5[:], in0=w1r_t[:], in1=w1r_t[:],
                                    op=ALU.mult)
            nc.vector.tensor_scalar(out=a5[:], in0=a5[:], scalar1=var5[:, 0:1],
                                    scalar2=None, op0=ALU.mult)
            nc.scalar.activation(a5[:], a5[:], Sqrt, bias=eps128[:BANDS, 0:1])
            nc.vector.reciprocal(a5[:], a5[:])
            nc.vector.tensor_tensor(out=a5[:], in0=a5[:], in1=w1r_t[:], op=ALU.mult)
            nc.vector.tensor_tensor(out=a5[:], in0=a5[:], in1=g1w_t[:], op=ALU.mult)
            # apm [5, 4, 32]: cols (k, sign): relu(a), relu(-a), relu(a), relu(-a)
            apm = cp.tile([BANDS, 4, 32], f32)
            nc.vector.tensor_copy(out=apm[:, 0:1, :], in_=a5[:])
            nc.vector.tensor_scalar(out=apm[:, 1:2, :], in0=a5[:],
                                    scalar1=-1.0, scalar2=None, op0=ALU.mult)
            nc.vector.tensor_copy(out=apm[:, 2:3, :], in_=apm[:, 0:1, :])
            nc.vector.tensor_copy(out=apm[:, 3:4, :], in_=apm[:, 1:2, :])
            nc.scalar.activation(apm[:], apm[:], Relu)
            # pq [5, 2, 2]: (band, k, sign): P=sum_j relu(a)W2, Q=sum_j relu(-a)W2
            prod = cp.tile([BANDS, 4, 32], f32)
            nc.vector.tensor_tensor(out=prod[:], in0=w2k_t[:], in1=apm[:],
                                    op=ALU.mult)
            pq = cp.tile([BANDS, 2, 2], f32)
            nc.vector.tensor_reduce(out=pq[:], in_=prod[:], axis=AX.X, op=ALU.add)
            p_ap = pq[:, :, 0:1]
            q_ap = pq[:, :, 1:2]

            # mu1 broadcast across partitions for z = s - mu
            mu_bc = cp.tile([128, BANDS], f32)
            nc.sync.dma_start(mu_bc[:],
                              cc1_out[:, 0:BANDS].to_broadcast([128, BANDS]))
            nc.vector.tensor_scalar(out=mu_bc[:], in0=mu_bc[:], scalar1=inv_n,
                                    scalar2=None, op0=ALU.mult)

            # ---------- L2: u = relu(+-z); v = As_blockdiag @ u ----------
            u_t = bp.tile([P114, nblk, 2 * BANDS], bf16)
            nc.vector.tensor_tensor(
                out=u_t[:, :, 0:BANDS], in0=s_t[:P114],
                in1=mu_bc[:P114, None, :].to_broadcast([P114, nblk, BANDS]),
                op=ALU.subtract,
            )
            nc.vector.tensor_scalar(out=u_t[:, :, BANDS : 2 * BANDS],
                                    in0=u_t[:, :, 0:BANDS], scalar1=-1.0,
                                    scalar2=None, op0=ALU.mult)
            nc.scalar.activation(u_t[:], u_t[:], Relu)
            if npad:
                # zero pad-slot rows via DMA (engines can't start at part. 76)
                nc.sync.dma_start(
                    u_t[(SLOT - npad) * NN : P114, nblk - 1, :],
                    zpad[: npad * NN, :])

            v_t = bp.tile([128, nblk, 2 * BANDS], bf16)
            nch2 = (nblk + CH2 - 1) // CH2
            with tc.tile_pool(name="ps2", bufs=4, space="PSUM") as pp2:
                for c in range(nch2):
                    c0 = c * CH2
                    nb = min(CH2, nblk - c0)
                    ps = pp2.tile([128, CH2, 2 * BANDS], f32, tag="ps2")
                    nc.tensor.matmul(
                        ps[:, :nb, :], as_t[:], u_t[:, c0 : c0 + nb, :],
                        start=True, stop=True,
                    )
                    nc.vector.tensor_copy(out=v_t[:, c0 : c0 + nb, :],
                                          in_=ps[:, :nb, :])

                # ---------- L2 stats ----------
                part2 = cp.tile([128, 25], f32)
                for b in range(BANDS):
                    nc.vector.tensor_reduce(out=part2[:, b : b + 1],
                                            in_=v_t[:, :, b], axis=AX.X, op=ALU.add)
                    nc.vector.tensor_reduce(out=part2[:, 5 + b : 6 + b],
                                            in_=v_t[:, :, 5 + b], axis=AX.X,
                                            op=ALU.add)
                    for k, (i0, i1) in ((10, (b, b)), (15, (5 + b, 5 + b)),
                                        (20, (b, 5 + b))):
                        scr = sp.tile([128, nblk], f32, tag="scr")
                        nc.vector.tensor_tensor(out=scr[:], in0=v_t[:, :, i0],
                                                in1=v_t[:, :, i1], op=ALU.mult)
                        nc.vector.tensor_reduce(out=part2[:, k + b : k + b + 1],
                                                in_=scr[:], axis=AX.X, op=ALU.add)
                pst2 = pp2.tile([25, 1], f32, tag="pst2")
                nc.tensor.matmul(pst2[:], part2[:], ones_t[:], start=True, stop=True)
                st2_t = cp.tile([25, 1], f32)
                nc.vector.tensor_copy(out=st2_t[:], in_=pst2[:])

            nc.sync.dma_start(cc2_in[:], st2_t[:])
            nc.gpsimd.collective_compute(
                "AllReduce", ALU.add, replica_groups=rgroups,
                ins=[cc2_in[:].opt()], outs=[cc2_out[:].opt()],
            )

            # ---------- BN2 coefficients: A,B,C [5, 2(k)] each ----------
            # stm5 [5, 5]: cols = (mvp, mvm, E[v+^2], E[v-^2], E[v+v-])
            stm5 = cp.tile([BANDS, 5], f32)
            for gidx in range(5):
                nc.sync.dma_start(
                    stm5[:, gidx : gidx + 1],
                    cc2_out[:, gidx * BANDS : (gidx + 1) * BANDS]
                    .rearrange("one b -> b one"))
            nc.vector.tensor_scalar(out=stm5[:], in0=stm5[:], scalar1=inv_n,
                                    scalar2=None, op0=ALU.mult)
            # mom [5, 3]: vvp, vvm, cvpm
            mom = cp.tile([BANDS, 3], f32)
            nc.vector.tensor_tensor(out=mom[:, 0:1], in0=stm5[:, 0:1],
                                    in1=stm5[:, 0:1], op=ALU.mult)
            nc.vector.tensor_tensor(out=mom[:, 0:1], in0=stm5[:, 2:3],
                                    in1=mom[:, 0:1], op=ALU.subtract)
            nc.vector.tensor_tensor(out=mom[:, 1:2], in0=stm5[:, 1:2],
                                    in1=stm5[:, 1:2], op=ALU.mult)
            nc.vector.tensor_tensor(out=mom[:, 1:2], in0=stm5[:, 3:4],
                                    in1=mom[:, 1:2], op=ALU.subtract)
            nc.vector.tensor_tensor(out=mom[:, 2:3], in0=stm5[:, 0:1],
                                    in1=stm5[:, 1:2], op=ALU.mult)
            nc.vector.tensor_tensor(out=mom[:, 2:3], in0=stm5[:, 4:5],
                                    in1=mom[:, 2:3], op=ALU.subtract)
            # mu2 = P*mvp + Q*mvm + b2
            mu2 = cp.tile([BANDS, 2], f32)
            t2a = cp.tile([BANDS, 2], f32)
            nc.vector.tensor_scalar(out=mu2[:], in0=p_ap, scalar1=stm5[:, 0:1],
                                    scalar2=None, op0=ALU.mult)
            nc.vector.tensor_scalar(out=t2a[:], in0=q_ap, scalar1=stm5[:, 1:2],
                                    scalar2=None, op0=ALU.mult)
            nc.vector.tensor_tensor(out=mu2[:], in0=mu2[:], in1=t2a[:], op=ALU.add)
            nc.vector.tensor_tensor(out=mu2[:], in0=mu2[:], in1=cv_t[:, 0:2],
                                    op=ALU.add)
            # var2 = P^2 vvp + Q^2 vvm + 2PQ cvpm
            var2 = cp.tile([BANDS, 2], f32)
            nc.vector.tensor_tensor(out=var2[:], in0=p_ap, in1=p_ap, op=ALU.mult)
            nc.vector.tensor_scalar(out=var2[:], in0=var2[:], scalar1=mom[:, 0:1],
                                    scalar2=None, op0=ALU.mult)
            nc.vector.tensor_tensor(out=t2a[:], in0=q_ap, in1=q_ap, op=ALU.mult)
            nc.vector.tensor_scalar(out=t2a[:], in0=t2a[:], scalar1=mom[:, 1:2],
                                    scalar2=None, op0=ALU.mult)
            nc.vector.tensor_tensor(out=var2[:], in0=var2[:], in1=t2a[:], op=ALU.add)
            nc.vector.tensor_tensor(out=t2a[:], in0=p_ap, in1=q_ap, op=ALU.mult)
            nc.vector.tensor_scalar(out=t2a[:], in0=t2a[:], scalar1=mom[:, 2:3],
                                    scalar2=None, op0=ALU.mult)
            nc.vector.tensor_scalar(out=t2a[:], in0=t2a[:], scalar1=2.0,
                                    scalar2=None, op0=ALU.mult)
            nc.vector.tensor_tensor(out=var2[:], in0=var2[:], in1=t2a[:], op=ALU.add)
            rs2 = cp.tile([BANDS, 2], f32)
            nc.scalar.activation(rs2[:], var2[:], Sqrt, bias=eps128[:BANDS, 0:1])
            nc.vector.reciprocal(rs2[:], rs2[:])
            nc.vector.tensor_tensor(out=rs2[:], in0=rs2[:], in1=cv_t[:, 2:4],
                                    op=ALU.mult)          # rs2 = rsqrt(var+eps)*g2
            # A = P*rs2, B = Q*rs2, C = (b2 - mu2)*rs2 + bt2   (abc cols: 2c + k)
            abc = cp.tile([BANDS, 6], f32)
            nc.vector.tensor_tensor(out=abc[:, 0:2], in0=p_ap, in1=rs2[:],
                                    op=ALU.mult)
            nc.vector.tensor_tensor(out=abc[:, 2:4], in0=q_ap, in1=rs2[:],
                                    op=ALU.mult)
            nc.vector.tensor_tensor(out=t2a[:], in0=cv_t[:, 0:2], in1=mu2[:],
                                    op=ALU.subtract)
            nc.vector.tensor_tensor(out=t2a[:], in0=t2a[:], in1=rs2[:], op=ALU.mult)
            nc.vector.tensor_tensor(out=abc[:, 4:6], in0=t2a[:], in1=cv_t[:, 4:6],
                                    op=ALU.add)
            # bounce through DRAM: abc_d col = c*10 + k*5 + b
            for ci in range(3):
                nc.sync.dma_start(
                    abc_d[:, ci * 10 : (ci + 1) * 10]
                    .rearrange("one (k b) -> b (one k)", k=2),
                    abc[:, 2 * ci : 2 * ci + 2])
            coef_bc = cp.tile([128, 30], f32)
            nc.sync.dma_start(coef_bc[:], abc_d[:].to_broadcast([128, 30]))

            # ---------- L3: xc = relu(A*v+ + B*v- + C); y1 = lin1(xc) ----------
            y1_t = bp.tile([128, nblk, SLOT], f32)
            with tc.tile_pool(name="ps3", bufs=4, space="PSUM") as pp3:
                for s in range(SLOT):
                    vs = sp.tile([NN, nblk, 2 * BANDS], bf16, tag="vs")
                    nc.sync.dma_start(vs[:], v_t[s * NN : (s + 1) * NN, :, :])
                    xcs = sp.tile([NN, nblk, 2 * BANDS], bf16, tag="xcs")
                    for k in range(2):
                        ksl = slice(k * BANDS, (k + 1) * BANDS)
                        scrb = sp1.tile([NN, nblk, BANDS], bf16, tag="scrb")
                        nc.vector.tensor_tensor(
                            out=xcs[:, :, ksl], in0=vs[:, :, 0:BANDS],
                            in1=coef_bc[:NN, None, k * BANDS : (k + 1) * BANDS]
                            .to_broadcast([NN, nblk, BANDS]),
                            op=ALU.mult,
                        )
                        nc.vector.tensor_tensor(
                            out=scrb[:], in0=vs[:, :, BANDS : 2 * BANDS],
                            in1=coef_bc[:NN, None, 10 + k * BANDS : 10 + (k + 1) * BANDS]
                            .to_broadcast([NN, nblk, BANDS]),
                            op=ALU.mult,
                        )
                        nc.vector.tensor_tensor(out=xcs[:, :, ksl],
                                                in0=xcs[:, :, ksl],
                                                in1=scrb[:], op=ALU.add)
                        nc.vector.tensor_tensor(
                            out=xcs[:, :, ksl], in0=xcs[:, :, ksl],
                            in1=coef_bc[:NN, None, 20 + k * BANDS : 20 + (k + 1) * BANDS]
                            .to_broadcast([NN, nblk, BANDS]),
                            op=ALU.add,
                        )
                    nc.scalar.activation(xcs[:], xcs[:], Relu)
                    for c0 in range(0, nblk, 512):
                        nb = min(512, nblk - c0)
                        ps = pp3.tile([128, 512], f32, tag="ps3")
                        for j in range(10):
                            nc.tensor.matmul(
                                ps[:, :nb], w1s_t[:, j, :],
                                xcs[:, c0 : c0 + nb, j],
                                start=(j == 0), stop=(j == 9),
                            )
                        nc.vector.tensor_scalar(
                            out=y1_t[:, c0 : c0 + nb, s], in0=ps[:, :nb],
                            scalar1=l1b_t[:, 0:1], scalar2=None, op0=ALU.add)
            if npad:
                nc.vector.memset(y1_t[:, nblk - 1, SLOT - npad : SLOT], 0.0)

            # ---------- BN3 stats ----------
            part3 = cp.tile([128, 2], f32)
            nc.vector.tensor_reduce(out=part3[:, 0:1], in_=y1_t[:], axis=AX.XY,
                                    op=ALU.add)
            x2_t = bp.tile([128, nblk, SLOT], bf16)
            nc.vector.tensor_tensor(out=x2_t[:], in0=y1_t[:], in1=y1_t[:],
                                    op=ALU.mult)
            nc.vector.tensor_reduce(out=part3[:, 1:2], in_=x2_t[:], axis=AX.XY,
                                    op=ALU.add)
            nc.sync.dma_start(cc3_in[:], part3[:])
            nc.gpsimd.collective_compute(
                "AllReduce", ALU.add, replica_groups=rgroups,
                ins=[cc3_in[:].opt()], outs=[cc3_out[:].opt()],
            )

            # ---------- BN3 coefficients + head ----------
            st3_t = cp.tile([128, 2], f32)
            nc.sync.dma_start(st3_t[:], cc3_out[:])
            nc.vector.tensor_scalar(out=st3_t[:], in0=st3_t[:], scalar1=inv_b,
                                    scalar2=None, op0=ALU.mult)
            g3c = cp.tile([128, 2], f32)        # col0 = G3, col1 = B3
            nc.vector.tensor_tensor(out=g3c[:, 0:1], in0=st3_t[:, 0:1],
                                    in1=st3_t[:, 0:1], op=ALU.mult)
            nc.vector.tensor_tensor(out=g3c[:, 0:1], in0=st3_t[:, 1:2],
                                    in1=g3c[:, 0:1], op=ALU.subtract)
            nc.scalar.activation(g3c[:, 0:1], g3c[:, 0:1], Sqrt,
                                 bias=eps128[:, 0:1])
            nc.vector.reciprocal(g3c[:, 0:1], g3c[:, 0:1])
            nc.vector.tensor_tensor(out=g3c[:, 0:1], in0=g3c[:, 0:1],
                                    in1=l1b_t[:, 1:2], op=ALU.mult)
            nc.vector.tensor_tensor(out=g3c[:, 1:2], in0=st3_t[:, 0:1],
                                    in1=g3c[:, 0:1], op=ALU.mult)
            nc.vector.tensor_tensor(out=g3c[:, 1:2], in0=l1b_t[:, 2:3],
                                    in1=g3c[:, 1:2], op=ALU.subtract)
            nc.vector.tensor_scalar(out=x2_t[:], in0=y1_t[:], scalar1=g3c[:, 0:1],
                                    scalar2=None, op0=ALU.mult)
            nc.scalar.activation(x2_t[:], x2_t[:], Relu, bias=g3c[:, 1:2])

            x3_t = bp.tile([32, nslot], bf16)
            x2f = x2_t[:].rearrange("p a b -> p (a b)")
            with tc.tile_pool(name="ps4", bufs=4, space="PSUM") as pp4:
                for c0 in range(0, nslot, 512):
                    nb = min(512, nslot - c0)
                    ps4 = pp4.tile([32, 512], f32, tag="ps4")
                    nc.tensor.matmul(ps4[:, :nb], w2l_t[:], x2f[:, c0 : c0 + nb],
                                     start=True, stop=True)
                    nc.scalar.activation(x3_t[:, c0 : c0 + nb], ps4[:, :nb], Relu,
                                         bias=l2b_t[:, 0:1])
                for c0 in range(0, nslot, 512):
                    nb = min(512, nslot - c0)
                    ps5 = pp4.tile([2, 512], f32, tag="ps5")
                    nc.tensor.matmul(ps5[:, :nb], w3l_t[:], x3_t[:, c0 : c0 + nb],
                                     start=True, stop=True)
                    yos = sp.tile([2, 512], f32, tag="yos")
                    nc.vector.tensor_scalar(out=yos[:, :nb], in0=ps5[:, :nb],
                                            scalar1=l3b_t[:, 0:1],
                                            scalar2=None, op0=ALU.add)
                    nc.sync.dma_start(yout[:, c0 : c0 + nb], yos[:, :nb])

    nc.compile()
    return nc


def _get_kernel():
    if "k" not in _KERNEL_CACHE:
        _KERNEL_CACHE["k"] = _build_fused((G + SLOT - 1) // SLOT,
                                          G, NCORES, N, B)
    return _KERNEL_CACHE["k"]


# --------------------------------------------------------------------------
# cached-jit runner (mirrors bass2jax.run_bass_via_pjrt, but caches the
# traced/compiled executable across calls)
# --------------------------------------------------------------------------
def _get_runner(nc, n_cores):
    key = id(nc)
    if key in _RUNNER_CACHE:
        return _RUNNER_CACHE[key]

    import jax
    from jax.experimental.shard_map import shard_map
    from jax.sharding import Mesh, PartitionSpec
    from concourse import bass2jax, mybir

    bass2jax.install_neuronx_cc_hook()
    assert nc.dbg_addr is None
    partition_name = nc.partition_id_tensor.name if nc.partition_id_tensor else None

    in_names, out_names, out_avals, zero_shapes = [], [], [], []
    for alloc in nc.m.functions[0].allocations:
        if not isinstance(alloc, mybir.MemoryLocationSet):
            continue
        name = alloc.memorylocations[0].name
        if alloc.kind == "ExternalInput":
            if name != partition_name:
                in_names.append(name)
        elif alloc.kind == "ExternalOutput":
            out_names.append(name)
            shape = tuple(alloc.tensor_shape)
            dtype = mybir.dt.np(alloc.dtype)
            out_avals.append(jax.core.ShapedArray(shape, dtype))
            zero_shapes.append((shape, dtype))
    n_params = len(in_names)
    all_in_names = (in_names + out_names
                    + ([partition_name] if partition_name else []))
    donate = tuple(range(n_params, n_params + len(out_names)))

    def _body(*args):
        operands = list(args)
        if partition_name is not None:
            operands.append(bass2jax.partition_id_tensor())
        outs = bass2jax._bass_exec_p.bind(
            *operands,
            out_avals=tuple(out_avals),
            in_names=tuple(all_in_names),
            out_names=tuple(out_names),
            lowering_input_output_aliases=(),
            sim_require_finite=True,
            sim_require_nnan=True,
            nc=nc,
        )
        return tuple(outs)

    devices = jax.devices()[:n_cores]
    assert len(devices) == n_cores
    mesh = Mesh(np.asarray(devices), ("core",))
    in_specs = (PartitionSpec("core"),) * (n_params + len(out_names))
    out_specs = (PartitionSpec("core"),) * len(out_names)
    sharded = jax.jit(
        shard_map(_body, mesh=mesh, in_specs=in_specs, out_specs=out_specs,
                  check_rep=False),
        donate_argnums=donate, keep_unused=True,
    )
    runner = (sharded, in_names, out_names, zero_shapes)
    _RUNNER_CACHE[key] = runner
    return runner


def _run(nc, in_maps, tag):
    n_cores = len(in_maps)
    sharded, in_names, out_names, zero_shapes = _get_runner(nc, n_cores)
    concat_in = [
        np.concatenate([np.asarray(in_maps[c][name]) for c in range(n_cores)],
                       axis=0)
        for name in in_names
    ]
    concat_zeros = [
        np.zeros((n_cores * shape[0], *shape[1:]), dtype)
        for shape, dtype in zero_shapes
    ]
    out_arrs = sharded(*concat_in, *concat_zeros)
    return [
        {
            name: np.asarray(out_arrs[i]).reshape(
                n_cores, *zero_shapes[i][0])[c]
            for i, name in enumerate(out_names)
        }
        for c in range(n_cores)
    ]


# --------------------------------------------------------------------------
# main entry
# --------------------------------------------------------------------------
def kernel(**inputs) -> np.ndarray:
    x = np.asarray(inputs["x"], np.float32)
    eif = np.asarray(inputs["edge_index_func"])
    eis = np.asarray(inputs["edge_index_struct"])
    ew = np.asarray(inputs["edge_weight_func"], np.float32)
    W1 = np.asarray(inputs["W1"], np.float32)
    g1 = np.asarray(inputs["g1"], np.float32)
    bt1 = np.asarray(inputs["bt1"], np.float32)
    W2 = np.asarray(inputs["W2"], np.float32)
    b2 = np.asarray(inputs["b2"], np.float32)
    g2 = np.asarray(inputs["g2"], np.float32)
    bt2 = np.asarray(inputs["bt2"], np.float32)
    lin1_W = np.asarray(inputs["lin1_W"], np.float32)
    lin1_b = np.asarray(inputs["lin1_b"], np.float32)
    g3 = np.asarray(inputs["g3"], np.float32)
    bt3 = np.asarray(inputs["bt3"], np.float32)
    lin2_W = np.asarray(inputs["lin2_W"], np.float32)
    lin2_b = np.asarray(inputs["lin2_b"], np.float32)
    lin3_W = np.asarray(inputs["lin3_W"], np.float32)
    lin3_b = np.asarray(inputs["lin3_b"], np.float32)

    sf, df = eif[0].astype(np.int64), eif[1].astype(np.int64)
    ss, ds = eis[0].astype(np.int64), eis[1].astype(np.int64)

    # --- structural-assumption checks (else exact numpy fallback) ---
    gs = ss // NN
    ok = np.array_equal(gs, ds // NN) and np.array_equal(
        gs, np.repeat(np.arange(B), ES)
    )
    gf = sf // NN
    ok = ok and np.array_equal(gf, df // NN) and np.array_equal(
        gf, np.repeat(np.arange(B), EF)
    )
    ssl, dsl = ss % NN, ds % NN
    ok = ok and np.array_equal(ssl.reshape(B, ES), np.broadcast_to(ssl[:ES], (B, ES)))
    ok = ok and np.array_equal(dsl.reshape(B, ES), np.broadcast_to(dsl[:ES], (B, ES)))
    ok = ok and np.abs(bt1).max() == 0.0
    if not ok:
        return _fallback_numpy(inputs)

    try:
        return _device_pipeline(x, sf, df, ew, ssl, dsl, W1, g1, W2, b2, g2, bt2,
                                lin1_W, lin1_b, g3, bt3, lin2_W, lin2_b,
                                lin3_W, lin3_b)
    except Exception as e:
        import traceback
        print(f"device pipeline failed ({e}); numpy fallback", file=sys.stderr)
        traceback.print_exc()
        return _fallback_numpy(inputs)


def _pack_host(x, sf, df, ew, ssl, dsl):
    """Build per-core packed x / func-adjacency and the shared structural
    block-diagonal matrix."""
    NBLK = (G + SLOT - 1) // SLOT
    NSLOT = NBLK * SLOT

    # normalized func adjacency, transposed, self-loop folded: [B, 19src, 19dst]
    deg_f = np.bincount(df, weights=ew.astype(np.float64), minlength=N) + 1.0
    dinv_f = (1.0 / np.sqrt(deg_f)).astype(np.float32)
    norm_f = dinv_f[sf] * ew * dinv_f[df]
    gf = sf // NN
    idx = gf * (NN * NN) + (sf % NN) * NN + (df % NN)
    AfT = np.bincount(idx, weights=norm_f.astype(np.float64),
                      minlength=B * NN * NN).astype(np.float32).reshape(B, NN, NN)
    dd = (dinv_f * dinv_f).reshape(B, NN)
    AfT[:, np.arange(NN), np.arange(NN)] += dd

    # shared structural adjacency (identical for all graphs)
    s0, d0 = ssl[:ES], dsl[:ES]
    deg_s = np.bincount(d0, minlength=NN).astype(np.float64) + 1.0
    dinv_s = 1.0 / np.sqrt(deg_s)
    AsT = np.zeros((NN, NN), np.float64)
    np.add.at(AsT, (s0, d0), dinv_s[s0] * dinv_s[d0])
    AsT[np.arange(NN), np.arange(NN)] += dinv_s * dinv_s
    asb = np.zeros((P114, 128), np.float32)
    for p in range(SLOT):
        asb[p * NN : (p + 1) * NN, p * NN : (p + 1) * NN] = AsT
    asb = asb.astype(BF16)

    x3 = x.reshape(B, NN, BANDS)
    xb_l, afp_l = [], []
    for c in range(NCORES):
        xs = np.zeros((NSLOT, NN, BANDS), np.float32)
        xs[:G] = x3[c * G : (c + 1) * G]
        xb = np.ascontiguousarray(
            xs.reshape(NBLK, SLOT, NN, BANDS).transpose(1, 2, 0, 3)
            .reshape(P114, NBLK, BANDS)).astype(BF16)
        Ac = np.zeros((NSLOT, NN, NN), np.float32)
        Ac[:G] = AfT[c * G : (c + 1) * G]
        afp = np.ascontiguousarray(
            Ac.reshape(NBLK, SLOT, NN, NN).transpose(1, 2, 0, 3)
            .reshape(P114, NBLK, NN)).astype(BF16)
        xb_l.append(xb)
        afp_l.append(afp)
    return xb_l, afp_l, asb, NBLK, NSLOT


def _small_weights(W1, g1, W2, b2, g2, bt2, lin1_W, lin1_b, g3, bt3,
                   lin2_W, lin2_b, lin3_W, lin3_b):
    w1r = np.ascontiguousarray(W1[:, 0, :])                    # [5, 32]
    # w2k [5, 4, 32]: col (k, sign) -> W2[b, :, k] (same for both signs)
    w2kk = np.empty((BANDS, 4, 32), np.float32)
    for k in range(2):
        for sgn in range(2):
            w2kk[:, k * 2 + sgn, :] = W2[:, :, k]
    cvec = np.ascontiguousarray(
        np.concatenate([b2, g2, bt2], axis=1))                 # [5, 6]
    # lin1 stationary per j=(k,b): w1s[n, j, :] = lin1_W[b*38 + n*2 + k]
    w1s = np.empty((NN, 10, 128), np.float32)
    for k in range(2):
        for b in range(BANDS):
            j = k * BANDS + b
            w1s[:, j, :] = lin1_W[b * 2 * NN + np.arange(NN) * 2 + k]
    w1s = w1s.astype(BF16)
    l1bv = np.ascontiguousarray(
        np.stack([lin1_b, g3, bt3], axis=1))                   # [128, 3]
    return {
        "w1r": w1r, "g1w": np.ascontiguousarray(g1),
        "w2k": w2kk, "cvec": cvec, "w1s": w1s, "l1bv": l1bv,
        "w2l": np.ascontiguousarray(lin2_W).astype(BF16),
        "l2b": lin2_b.reshape(32, 1),
        "w3l": np.ascontiguousarray(lin3_W).astype(BF16),
        "l3b": lin3_b.reshape(2, 1),
    }


def _device_pipeline(x, sf, df, ew, ssl, dsl, W1, g1, W2, b2, g2, bt2,
                     lin1_W, lin1_b, g3, bt3, lin2_W, lin2_b, lin3_W, lin3_b):
    xb_l, afp_l, asb, NBLK, NSLOT = _pack_host(x, sf, df, ew, ssl, dsl)
    common = {"asb": asb, **_small_weights(W1, g1, W2, b2, g2, bt2, lin1_W,
                                           lin1_b, g3, bt3, lin2_W, lin2_b,
                                           lin3_W, lin3_b)}
    in_maps = [{"xb": xb_l[c], "afp": afp_l[c], **common} for c in range(NCORES)]

    nc = _get_kernel()
    res = _run(nc, in_maps, "fused")

    out = np.empty((B, 2), np.float32)
    for c in range(NCORES):
        out[c * G : (c + 1) * G] = res[c]["yout"][:, :G].T
    return out


# revision 19
# speedup vs baseline: 39.6751x; 1.6619x over previous
"""
Trainium2 Bass kernel for nn_MF_MGCN (5-band 2-layer GCN + MLP head).

Single fused device launch (vs the 4-launch baseline):
  * BatchNorm statistics are reduced across the 8 cores with on-device
    AllReduce collectives (3x tiny: 10, 25, and 256 floats), and the BN
    coefficient algebra runs on-device, so no host round trips remain.
  * The func adjacency ships packed as [114, NBLK, 19] bf16 (~3MB/core
    instead of ~20MB/core block-diagonal); the block-diagonal stationary
    tiles are assembled in SBUF by strided DMAs over a zeroed background.
  * Math identical to the baseline derivation: GCN1 on a 1-channel input
    is rank-1, so relu(BN1) @ W2 collapses onto (relu(z), relu(-z)) and
    GCN2 aggregates just 2 channels/band through one shared structural
    block-diagonal matrix.
  * The executable (jit of shard_map over 8 cores) is cached across
    calls, so warm runs skip retrace/recompile.

All compute-engine operands keep partition base 0 (hardware restricts
bases to {0,32,64}); any partition reshuffling goes through DMA.

If structural assumptions fail (shared struct pattern, bt1 == 0), a pure
numpy fallback reproduces the reference exactly.
"""

import sys

sys.path.insert(0, "/opt/trn_rl_repo")

import numpy as np
import ml_dtypes

BF16 = ml_dtypes.bfloat16

# Problem constants (hardcoded per task contract).
B = 32768
NN = 19
N = B * NN
BANDS = 5
EF, ES = 120, 60
EPS = 1e-5
NCORES = 8
G = B // NCORES           # graphs per core = 4096
SLOT = 6                  # graphs per 114-row block
P114 = SLOT * NN          # 114

_KERNEL_CACHE = {}
_RUNNER_CACHE = {}


# --------------------------------------------------------------------------
# numpy fallback (exact reference math)
# --------------------------------------------------------------------------
def _bn_np(h, g, b):
    m = h.mean(0)
    v = h.var(0)
    return (h - m) / np.sqrt(v + EPS) * g + b


def _gcn_np(h, W, b, src, dst, ew, n):
    h = h @ W
    deg = np.zeros(n, np.float64)
    np.add.at(deg, dst, ew)
    deg += 1.0
    dinv = 1.0 / np.sqrt(deg)
    norm = dinv[src] * ew * dinv[dst]
    agg = np.zeros_like(h, dtype=np.float64)
    np.add.at(agg, dst, norm[:, None] * h[src])
    return agg + (dinv * dinv)[:, None] * h + b


def _fallback_numpy(i):
    x = np.asarray(i["x"], np.float64)
    sf, df = np.asarray(i["edge_index_func"][0]), np.asarray(i["edge_index_func"][1])
    ss, ds = np.asarray(i["edge_index_struct"][0]), np.asarray(i["edge_index_struct"][1])
    ew = np.asarray(i["edge_weight_func"], np.float64)
    ews = np.ones(ss.shape[0], np.float64)
    n = x.shape[0]
    outs = []
    for b in range(BANDS):
        h = _gcn_np(x[:, b : b + 1], np.asarray(i["W1"][b], np.float64),
                    np.asarray(i["b1"][b], np.float64), sf, df, ew, n)
        h = np.maximum(_bn_np(h, np.asarray(i["g1"][b], np.float64),
                              np.asarray(i["bt1"][b], np.float64)), 0)
        h = _gcn_np(h, np.asarray(i["W2"][b], np.float64),
                    np.asarray(i["b2"][b], np.float64), ss, ds, ews, n)
        h = np.maximum(_bn_np(h, np.asarray(i["g2"][b], np.float64),
                              np.asarray(i["bt2"][b], np.float64)), 0)
        outs.append(h.reshape(n // NN, NN * 2))
    xc = np.concatenate(outs, axis=1)
    h = np.maximum(_bn_np(xc @ np.asarray(i["lin1_W"], np.float64)
                          + np.asarray(i["lin1_b"], np.float64),
                          np.asarray(i["g3"], np.float64),
                          np.asarray(i["bt3"], np.float64)), 0)
    h = np.maximum(h @ np.asarray(i["lin2_W"], np.float64)
                   + np.asarray(i["lin2_b"], np.float64), 0)
    out = h @ np.asarray(i["lin3_W"], np.float64) + np.asarray(i["lin3_b"], np.float64)
    return out.astype(np.float32)


# --------------------------------------------------------------------------
# fused Bass kernel builder
# --------------------------------------------------------------------------
def _build_fused(nblk, g_per_core, ncores, n_total, b_total):
    import concourse.bacc as bacc
    import concourse.mybir as mybir
    from concourse import tile

    f32, bf16 = mybir.dt.float32, mybir.dt.bfloat16
    Relu = mybir.ActivationFunctionType.Relu
    Sqrt = mybir.ActivationFunctionType.Sqrt
    ALU = mybir.AluOpType
    AX = mybir.AxisListType

    nslot = nblk * SLOT
    npad = nslot - g_per_core          # pad graph slots (live in last block)
    assert 0 <= npad < SLOT
    CH1 = min(48, nblk)                # L1 blocks per chunk
    CH2 = min(51, nblk)                # L2 blocks per chunk (510 moving cols)
    rgroups = [list(range(ncores))]

    nc = bacc.Bacc(None, target_bir_lowering=False, num_devices=ncores)

    xb = nc.dram_tensor("xb", [P114, nblk, BANDS], bf16, kind="ExternalInput")
    afp = nc.dram_tensor("afp", [P114, nblk, NN], bf16, kind="ExternalInput")
    asb = nc.dram_tensor("asb", [P114, 128], bf16, kind="ExternalInput")
    w1r = nc.dram_tensor("w1r", [BANDS, 32], f32, kind="ExternalInput")
    g1w = nc.dram_tensor("g1w", [BANDS, 32], f32, kind="ExternalInput")
    w2k = nc.dram_tensor("w2k", [BANDS, 4, 32], f32, kind="ExternalInput")
    cvec = nc.dram_tensor("cvec", [BANDS, 6], f32, kind="ExternalInput")
    w1s = nc.dram_tensor("w1s", [NN, 10, 128], f32, kind="ExternalInput")
    l1bv = nc.dram_tensor("l1bv", [128, 3], f32, kind="ExternalInput")
    w2l = nc.dram_tensor("w2l", [128, 32], f32, kind="ExternalInput")
    l2b = nc.dram_tensor("l2b", [32, 1], f32, kind="ExternalInput")
    w3l = nc.dram_tensor("w3l", [32, 2], f32, kind="ExternalInput")
    l3b = nc.dram_tensor("l3b", [2, 1], f32, kind="ExternalInput")
    yout = nc.dram_tensor("yout", [2, nslot], f32, kind="ExternalOutput")

    inv_n = 1.0 / float(n_total)
    inv_b = 1.0 / float(b_total)

    with tile.TileContext(nc) as tc:
        with (
            tc.tile_pool(name="const", bufs=1) as cp,
            tc.tile_pool(name="big", bufs=1) as bp,
            tc.tile_pool(name="scr", bufs=2) as sp,
            tc.tile_pool(name="scr1", bufs=1) as sp1,
            tc.tile_pool(name="dram", bufs=1, space="DRAM") as dp,
        ):
            # ---------- persistent small loads ----------
            as_t = cp.tile([P114, 128], bf16)
            nc.sync.dma_start(as_t[:], asb[:])
            w1r_t = cp.tile([BANDS, 32], f32)
            nc.sync.dma_start(w1r_t[:], w1r[:])
            g1w_t = cp.tile([BANDS, 32], f32)
            nc.sync.dma_start(g1w_t[:], g1w[:])
            w2k_t = cp.tile([BANDS, 4, 32], f32)
            nc.sync.dma_start(w2k_t[:], w2k[:])
            cv_t = cp.tile([BANDS, 6], f32)
            nc.sync.dma_start(cv_t[:], cvec[:])
            w1s_t = cp.tile([NN, 10, 128], f32)
            nc.sync.dma_start(w1s_t[:], w1s[:])
            l1b_t = cp.tile([128, 3], f32)
            nc.sync.dma_start(l1b_t[:], l1bv[:])
            w2l_t = cp.tile([128, 32], f32)
            nc.sync.dma_start(w2l_t[:], w2l[:])
            l2b_t = cp.tile([32, 1], f32)
            nc.sync.dma_start(l2b_t[:], l2b[:])
            w3l_t = cp.tile([32, 2], f32)
            nc.sync.dma_start(w3l_t[:], w3l[:])
            l3b_t = cp.tile([2, 1], f32)
            nc.sync.dma_start(l3b_t[:], l3b[:])
            ones_t = cp.tile([128, 1], f32)
            nc.vector.memset(ones_t[:], 1.0)
            eps128 = cp.tile([128, 1], f32)
            nc.vector.memset(eps128[:], EPS)
            zpad = cp.tile([P114 - NN, 2 * BANDS], bf16)
            nc.vector.memset(zpad[:], 0.0)

            # DRAM bounce buffers for collectives
            cc1_in = dp.tile([10, 1], f32)
            cc1_out = dp.tile([1, 10], f32)
            cc2_in = dp.tile([25, 1], f32)
            cc2_out = dp.tile([1, 25], f32)
            cc3_in = dp.tile([128, 2], f32)
            cc3_out = dp.tile([128, 2], f32)
            abc_d = dp.tile([1, 30], f32)

            y1_t = bp.tile([128, nblk, SLOT], f32)

            with tc.tile_pool(name="pv", bufs=1) as pv:
                v_t = pv.tile([128, nblk, 2 * BANDS], f32)

                with tc.tile_pool(name="pu", bufs=1) as pu:
                    u_t = pu.tile([P114, nblk, 2 * BANDS], bf16)

                    # ---------- L1: s = AfT_blockdiag @ x ----------
                    with tc.tile_pool(name="p1", bufs=1) as p1:
                        x_t = p1.tile([P114, nblk, BANDS], bf16)
                        nc.sync.dma_start(x_t[:], xb[:])
                        s_t = p1.tile([128, nblk, BANDS], f32)
                        a0 = p1.tile([P114, CH1, 128], bf16)
                        a1 = p1.tile([P114, CH1, 128], bf16)
                        nc.vector.memset(a0[:], 0.0)
                        nc.vector.memset(a1[:], 0.0)
                        a_bufs = [a0, a1]
                        nch1 = (nblk + CH1 - 1) // CH1
                        with tc.tile_pool(name="ps1", bufs=4, space="PSUM") as pp1:
                            for c in range(nch1):
                                c0 = c * CH1
                                nb = min(CH1, nblk - c0)
                                a_t = a_bufs[c % 2]
                                for p in range(SLOT):
                                    nc.sync.dma_start(
                                        a_t[p * NN : (p + 1) * NN, :nb,
                                            p * NN : (p + 1) * NN],
                                        afp[p * NN : (p + 1) * NN,
                                            c0 : c0 + nb, :],
                                    )
                                ps = pp1.tile([128, CH1, BANDS], f32, tag="ps1")
                                for j in range(nb):
                                    nc.tensor.matmul(
                                        ps[:, j, :], a_t[:, j, :], x_t[:, c0 + j, :],
                                        start=True, stop=True,
                                    )
                                nc.vector.tensor_copy(out=s_t[:, c0 : c0 + nb, :],
                                                      in_=ps[:, :nb, :])

                            # ---------- L1 stats: sum(s), sum(s^2) per band ----
                            part1 = cp.tile([128, 10], f32)
                            for b in range(BANDS):
                                nc.vector.tensor_reduce(
                                    out=part1[:, b : b + 1], in_=s_t[:, :, b],
                                    axis=AX.X, op=ALU.add)
                                scr = sp.tile([128, nblk], f32, tag="scr")
                                nc.vector.tensor_tensor(
                                    out=scr[:], in0=s_t[:, :, b],
                                    in1=s_t[:, :, b], op=ALU.mult)
                                nc.vector.tensor_reduce(
                                    out=part1[:, 5 + b : 6 + b], in_=scr[:],
                                    axis=AX.X, op=ALU.add)
                            pst1 = pp1.tile([10, 1], f32, tag="pst1")
                            nc.tensor.matmul(pst1[:], part1[:], ones_t[:],
                                             start=True, stop=True)
                            st1_t = cp.tile([10, 1], f32)
                            nc.vector.tensor_copy(out=st1_t[:], in_=pst1[:])

                        nc.sync.dma_start(cc1_in[:], st1_t[:])
                        nc.gpsimd.collective_compute(
                            "AllReduce", ALU.add, replica_groups=rgroups,
                            ins=[cc1_in[:].opt()], outs=[cc1_out[:].opt()],
                        )

                        # ---------- BN1 coefficients ----------
                        mv_t = cp.tile([BANDS, 2], f32)   # col0 mu1, col1 E[s^2]
                        nc.sync.dma_start(
                            mv_t[:, 0:1],
                            cc1_out[:, 0:BANDS].rearrange("one b -> b one"))
                        nc.sync.dma_start(
                            mv_t[:, 1:2],
                            cc1_out[:, BANDS : 2 * BANDS]
                            .rearrange("one b -> b one"))
                        nc.vector.tensor_scalar(out=mv_t[:], in0=mv_t[:],
                                                scalar1=inv_n, scalar2=None,
                                                op0=ALU.mult)
                        var5 = cp.tile([BANDS, 1], f32)
                        nc.vector.tensor_tensor(out=var5[:], in0=mv_t[:, 0:1],
                                                in1=mv_t[:, 0:1], op=ALU.mult)
                        nc.vector.tensor_tensor(out=var5[:], in0=mv_t[:, 1:2],
                                                in1=var5[:], op=ALU.subtract)
                        # a = w1r * rsqrt(var*w1r^2 + eps) * g1   [5, 32]
                        a5 = cp.tile([BANDS, 32], f32)
                        nc.vector.tensor_tensor(out=a5[:], in0=w1r_t[:],
                                                in1=w1r_t[:], op=ALU.mult)
                        nc.vector.tensor_scalar(out=a5[:], in0=a5[:],
                                                scalar1=var5[:, 0:1],
                                                scalar2=None, op0=ALU.mult)
                        nc.scalar.activation(a5[:], a5[:], Sqrt,
                                             bias=eps128[:BANDS, 0:1])
                        nc.vector.reciprocal(a5[:], a5[:])
                        nc.vector.tensor_tensor(out=a5[:], in0=a5[:],
                                                in1=w1r_t[:], op=ALU.mult)
                        nc.vector.tensor_tensor(out=a5[:], in0=a5[:],
                                                in1=g1w_t[:], op=ALU.mult)
                        # apm [5, 4, 32] cols (k, sign)
                        apm = cp.tile([BANDS, 4, 32], f32)
                        nc.vector.tensor_copy(out=apm[:, 0:1, :], in_=a5[:])
                        nc.vector.tensor_scalar(out=apm[:, 1:2, :], in0=a5[:],
                                                scalar1=-1.0, scalar2=None,
                                                op0=ALU.mult)
                        nc.vector.tensor_copy(out=apm[:, 2:3, :],
                                              in_=apm[:, 0:1, :])
                        nc.vector.tensor_copy(out=apm[:, 3:4, :],
                                              in_=apm[:, 1:2, :])
                        nc.scalar.activation(apm[:], apm[:], Relu)
                        # pq [5, 2, 2]: (band, k, sign)
                        prod = cp.tile([BANDS, 4, 32], f32)
                        nc.vector.tensor_tensor(out=prod[:], in0=w2k_t[:],
                                                in1=apm[:], op=ALU.mult)
                        pq = cp.tile([BANDS, 2, 2], f32)
                        nc.vector.tensor_reduce(out=pq[:], in_=prod[:],
                                                axis=AX.X, op=ALU.add)
                        p_ap = pq[:, :, 0:1]
                        q_ap = pq[:, :, 1:2]

                        # mu1 broadcast across partitions for z = s - mu
                        mu_bc = cp.tile([128, BANDS], f32)
                        nc.sync.dma_start(
                            mu_bc[:],
                            cc1_out[:, 0:BANDS].to_broadcast([128, BANDS]))
                        nc.vector.tensor_scalar(out=mu_bc[:], in0=mu_bc[:],
                                                scalar1=inv_n, scalar2=None,
                                                op0=ALU.mult)

                        # ---------- u = relu(+-(s - mu)) ----------
                        nc.vector.tensor_tensor(
                            out=u_t[:, :, 0:BANDS], in0=s_t[:P114],
                            in1=mu_bc[:P114, None, :]
                            .to_broadcast([P114, nblk, BANDS]),
                            op=ALU.subtract,
                        )
                        nc.vector.tensor_scalar(
                            out=u_t[:, :, BANDS : 2 * BANDS],
                            in0=u_t[:, :, 0:BANDS], scalar1=-1.0,
                            scalar2=None, op0=ALU.mult)
                        nc.scalar.activation(u_t[:], u_t[:], Relu)
                        if npad:
                            # zero pad-slot rows via DMA (engine partition
                            # bases are restricted to {0,32,64})
                            nc.sync.dma_start(
                                u_t[(SLOT - npad) * NN : P114, nblk - 1, :],
                                zpad[: npad * NN, :])
                    # p1 closed: x_t / a / s_t freed

                    # ---------- L2: v = As_blockdiag @ u ----------
                    nch2 = (nblk + CH2 - 1) // CH2
                    with tc.tile_pool(name="ps2", bufs=4, space="PSUM") as pp2:
                        for c in range(nch2):
                            c0 = c * CH2
                            nb = min(CH2, nblk - c0)
                            ps = pp2.tile([128, CH2, 2 * BANDS], f32, tag="ps2")
                            nc.tensor.matmul(
                                ps[:, :nb, :], as_t[:], u_t[:, c0 : c0 + nb, :],
                                start=True, stop=True,
                            )
                            nc.vector.tensor_copy(out=v_t[:, c0 : c0 + nb, :],
                                                  in_=ps[:, :nb, :])

                        # ---------- L2 stats ----------
                        part2 = cp.tile([128, 25], f32)
                        for b in range(BANDS):
                            nc.vector.tensor_reduce(
                                out=part2[:, b : b + 1], in_=v_t[:, :, b],
                                axis=AX.X, op=ALU.add)
                            nc.vector.tensor_reduce(
                                out=part2[:, 5 + b : 6 + b],
                                in_=v_t[:, :, 5 + b], axis=AX.X, op=ALU.add)
                            for k, (i0, i1) in ((10, (b, b)),
                                                (15, (5 + b, 5 + b)),
                                                (20, (b, 5 + b))):
                                scr = sp.tile([128, nblk], f32, tag="scr")
                                nc.vector.tensor_tensor(
                                    out=scr[:], in0=v_t[:, :, i0],
                                    in1=v_t[:, :, i1], op=ALU.mult)
                                nc.vector.tensor_reduce(
                                    out=part2[:, k + b : k + b + 1],
                                    in_=scr[:], axis=AX.X, op=ALU.add)
                        pst2 = pp2.tile([25, 1], f32, tag="pst2")
                        nc.tensor.matmul(pst2[:], part2[:], ones_t[:],
                                         start=True, stop=True)
                        st2_t = cp.tile([25, 1], f32)
                        nc.vector.tensor_copy(out=st2_t[:], in_=pst2[:])

                    nc.sync.dma_start(cc2_in[:], st2_t[:])
                    nc.gpsimd.collective_compute(
                        "AllReduce", ALU.add, replica_groups=rgroups,
                        ins=[cc2_in[:].opt()], outs=[cc2_out[:].opt()],
                    )
                # pu closed: u_t freed

                # ---------- BN2 coefficients: A,B,C [5, 2(k)] ----------
                stm5 = cp.tile([BANDS, 5], f32)
                for gidx in range(5):
                    nc.sync.dma_start(
                        stm5[:, gidx : gidx + 1],
                        cc2_out[:, gidx * BANDS : (gidx + 1) * BANDS]
                        .rearrange("one b -> b one"))
                nc.vector.tensor_scalar(out=stm5[:], in0=stm5[:], scalar1=inv_n,
                                        scalar2=None, op0=ALU.mult)
                mom = cp.tile([BANDS, 3], f32)
                nc.vector.tensor_tensor(out=mom[:, 0:1], in0=stm5[:, 0:1],
                                        in1=stm5[:, 0:1], op=ALU.mult)
                nc.vector.tensor_tensor(out=mom[:, 0:1], in0=stm5[:, 2:3],
                                        in1=mom[:, 0:1], op=ALU.subtract)
                nc.vector.tensor_tensor(out=mom[:, 1:2], in0=stm5[:, 1:2],
                                        in1=stm5[:, 1:2], op=ALU.mult)
                nc.vector.tensor_tensor(out=mom[:, 1:2], in0=stm5[:, 3:4],
                                        in1=mom[:, 1:2], op=ALU.subtract)
                nc.vector.tensor_tensor(out=mom[:, 2:3], in0=stm5[:, 0:1],
                                        in1=stm5[:, 1:2], op=ALU.mult)
                nc.vector.tensor_tensor(out=mom[:, 2:3], in0=stm5[:, 4:5],
                                        in1=mom[:, 2:3], op=ALU.subtract)
                mu2 = cp.tile([BANDS, 2], f32)
                t2a = cp.tile([BANDS, 2], f32)
                nc.vector.tensor_scalar(out=mu2[:], in0=p_ap,
                                        scalar1=stm5[:, 0:1],
                                        scalar2=None, op0=ALU.mult)
                nc.vector.tensor_scalar(out=t2a[:], in0=q_ap,
                                        scalar1=stm5[:, 1:2],
                                        scalar2=None, op0=ALU.mult)
                nc.vector.tensor_tensor(out=mu2[:], in0=mu2[:], in1=t2a[:],
                                        op=ALU.add)
                nc.vector.tensor_tensor(out=mu2[:], in0=mu2[:], in1=cv_t[:, 0:2],
                                        op=ALU.add)
                var2 = cp.tile([BANDS, 2], f32)
                nc.vector.tensor_tensor(out=var2[:], in0=p_ap, in1=p_ap,
                                        op=ALU.mult)
                nc.vector.tensor_scalar(out=var2[:], in0=var2[:],
                                        scalar1=mom[:, 0:1],
                                        scalar2=None, op0=ALU.mult)
                nc.vector.tensor_tensor(out=t2a[:], in0=q_ap, in1=q_ap,
                                        op=ALU.mult)
                nc.vector.tensor_scalar(out=t2a[:], in0=t2a[:],
                                        scalar1=mom[:, 1:2],
                                        scalar2=None, op0=ALU.mult)
                nc.vector.tensor_tensor(out=var2[:], in0=var2[:], in1=t2a[:],
                                        op=ALU.add)
                nc.vector.tensor_tensor(out=t2a[:], in0=p_ap, in1=q_ap,
                                        op=ALU.mult)
                nc.vector.tensor_scalar(out=t2a[:], in0=t2a[:],
                                        scalar1=mom[:, 2:3],
                                        scalar2=None, op0=ALU.mult)
                nc.vector.tensor_scalar(out=t2a[:], in0=t2a[:], scalar1=2.0,
                                        scalar2=None, op0=ALU.mult)
                nc.vector.tensor_tensor(out=var2[:], in0=var2[:], in1=t2a[:],
                                        op=ALU.add)
                rs2 = cp.tile([BANDS, 2], f32)
                nc.scalar.activation(rs2[:], var2[:], Sqrt,
                                     bias=eps128[:BANDS, 0:1])
                nc.vector.reciprocal(rs2[:], rs2[:])
                nc.vector.tensor_tensor(out=rs2[:], in0=rs2[:], in1=cv_t[:, 2:4],
                                        op=ALU.mult)   # rsqrt(var+eps)*g2
                abc = cp.tile([BANDS, 6], f32)
                nc.vector.tensor_tensor(out=abc[:, 0:2], in0=p_ap, in1=rs2[:],
                                        op=ALU.mult)
                nc.vector.tensor_tensor(out=abc[:, 2:4], in0=q_ap, in1=rs2[:],
                                        op=ALU.mult)
                nc.vector.tensor_tensor(out=t2a[:], in0=cv_t[:, 0:2], in1=mu2[:],
                                        op=ALU.subtract)
                nc.vector.tensor_tensor(out=t2a[:], in0=t2a[:], in1=rs2[:],
                                        op=ALU.mult)
                nc.vector.tensor_tensor(out=abc[:, 4:6], in0=t2a[:],
                                        in1=cv_t[:, 4:6], op=ALU.add)
                for ci in range(3):
                    nc.sync.dma_start(
                        abc_d[:, ci * 10 : (ci + 1) * 10]
                        .rearrange("one (k b) -> b (one k)", k=2),
                        abc[:, 2 * ci : 2 * ci + 2])
                coef_bc = cp.tile([128, 30], f32)
                nc.sync.dma_start(coef_bc[:], abc_d[:].to_broadcast([128, 30]))

                # ---------- L3: xc = relu(A*v+ + B*v- + C); y1 = lin1(xc) ----
                with (
                    tc.tile_pool(name="p3", bufs=2) as p3,
                    tc.tile_pool(name="ps3", bufs=4, space="PSUM") as pp3,
                ):
                    for s in range(SLOT):
                        vs = p3.tile([NN, nblk, 2 * BANDS], f32, tag="vs")
                        nc.sync.dma_start(vs[:], v_t[s * NN : (s + 1) * NN, :, :])
                        xcs = p3.tile([NN, nblk, 2 * BANDS], f32, tag="xcs")
                        for k in range(2):
                            ksl = slice(k * BANDS, (k + 1) * BANDS)
                            scrb = sp1.tile([NN, nblk, BANDS], f32, tag="scrb")
                            nc.vector.tensor_tensor(
                                out=xcs[:, :, ksl], in0=vs[:, :, 0:BANDS],
                                in1=coef_bc[:NN, None,
                                            k * BANDS : (k + 1) * BANDS]
                                .to_broadcast([NN, nblk, BANDS]),
                                op=ALU.mult,
                            )
                            nc.vector.tensor_tensor(
                                out=scrb[:], in0=vs[:, :, BANDS : 2 * BANDS],
                                in1=coef_bc[:NN, None,
                                            10 + k * BANDS : 10 + (k + 1) * BANDS]
                                .to_broadcast([NN, nblk, BANDS]),
                                op=ALU.mult,
                            )
                            nc.vector.tensor_tensor(out=xcs[:, :, ksl],
                                                    in0=xcs[:, :, ksl],
                                                    in1=scrb[:], op=ALU.add)
                            nc.vector.tensor_tensor(
                                out=xcs[:, :, ksl], in0=xcs[:, :, ksl],
                                in1=coef_bc[:NN, None,
                                            20 + k * BANDS : 20 + (k + 1) * BANDS]
                                .to_broadcast([NN, nblk, BANDS]),
                                op=ALU.add,
                            )
                        nc.scalar.activation(xcs[:], xcs[:], Relu)
                        for c0 in range(0, nblk, 512):
                            nb = min(512, nblk - c0)
                            ps = pp3.tile([128, 512], f32, tag="ps3")
                            for j in range(10):
                                nc.tensor.matmul(
                                    ps[:, :nb], w1s_t[:, j, :],
                                    xcs[:, c0 : c0 + nb, j],
                                    start=(j == 0), stop=(j == 9),
                                )
                            nc.vector.tensor_scalar(
                                out=y1_t[:, c0 : c0 + nb, s], in0=ps[:, :nb],
                                scalar1=l1b_t[:, 0:1], scalar2=None,
                                op0=ALU.add)
            # pv closed: v_t freed
            if npad:
                nc.vector.memset(y1_t[:, nblk - 1, SLOT - npad : SLOT], 0.0)

            # ---------- BN3 stats + head ----------
            with tc.tile_pool(name="ph", bufs=1) as ph:
                part3 = cp.tile([128, 2], f32)
                nc.vector.tensor_reduce(out=part3[:, 0:1], in_=y1_t[:],
                                        axis=AX.XY, op=ALU.add)
                x2_t = ph.tile([128, nblk, SLOT], f32)
                nc.vector.tensor_tensor(out=x2_t[:], in0=y1_t[:], in1=y1_t[:],
                                        op=ALU.mult)
                nc.vector.tensor_reduce(out=part3[:, 1:2], in_=x2_t[:],
                                        axis=AX.XY, op=ALU.add)
                nc.sync.dma_start(cc3_in[:], part3[:])
                nc.gpsimd.collective_compute(
                    "AllReduce", ALU.add, replica_groups=rgroups,
                    ins=[cc3_in[:].opt()], outs=[cc3_out[:].opt()],
                )

                st3_t = cp.tile([128, 2], f32)
                nc.sync.dma_start(st3_t[:], cc3_out[:])
                nc.vector.tensor_scalar(out=st3_t[:], in0=st3_t[:],
                                        scalar1=inv_b, scalar2=None,
                                        op0=ALU.mult)
                g3c = cp.tile([128, 2], f32)        # col0 G3, col1 B3
                nc.vector.tensor_tensor(out=g3c[:, 0:1], in0=st3_t[:, 0:1],
                                        in1=st3_t[:, 0:1], op=ALU.mult)
                nc.vector.tensor_tensor(out=g3c[:, 0:1], in0=st3_t[:, 1:2],
                                        in1=g3c[:, 0:1], op=ALU.subtract)
                nc.scalar.activation(g3c[:, 0:1], g3c[:, 0:1], Sqrt,
                                     bias=eps128[:, 0:1])
                nc.vector.reciprocal(g3c[:, 0:1], g3c[:, 0:1])
                nc.vector.tensor_tensor(out=g3c[:, 0:1], in0=g3c[:, 0:1],
                                        in1=l1b_t[:, 1:2], op=ALU.mult)
                nc.vector.tensor_tensor(out=g3c[:, 1:2], in0=st3_t[:, 0:1],
                                        in1=g3c[:, 0:1], op=ALU.mult)
                nc.vector.tensor_tensor(out=g3c[:, 1:2], in0=l1b_t[:, 2:3],
                                        in1=g3c[:, 1:2], op=ALU.subtract)
                nc.vector.tensor_scalar(out=x2_t[:], in0=y1_t[:],
                                        scalar1=g3c[:, 0:1], scalar2=None,
                                        op0=ALU.mult)
                nc.scalar.activation(x2_t[:], x2_t[:], Relu, bias=g3c[:, 1:2])

                x3_t = ph.tile([32, nslot], f32)
                x2f = x2_t[:].rearrange("p a b -> p (a b)")
                with tc.tile_pool(name="ps4", bufs=4, space="PSUM") as pp4:
                    for c0 in range(0, nslot, 512):
                        nb = min(512, nslot - c0)
                        ps4 = pp4.tile([32, 512], f32, tag="ps4")
                        nc.tensor.matmul(ps4[:, :nb], w2l_t[:],
                                         x2f[:, c0 : c0 + nb],
                                         start=True, stop=True)
                        nc.scalar.activation(x3_t[:, c0 : c0 + nb], ps4[:, :nb],
                                             Relu, bias=l2b_t[:, 0:1])
                    for c0 in range(0, nslot, 512):
                        nb = min(512, nslot - c0)
                        ps5 = pp4.tile([2, 512], f32, tag="ps5")
                        nc.tensor.matmul(ps5[:, :nb], w3l_t[:],
                                         x3_t[:, c0 : c0 + nb],
                                         start=True, stop=True)
                        yos = sp.tile([2, 512], f32, tag="yos")
                        nc.vector.tensor_scalar(out=yos[:, :nb], in0=ps5[:, :nb],
                                                scalar1=l3b_t[:, 0:1],
                                                scalar2=None, op0=ALU.add)
                        nc.sync.dma_start(yout[:, c0 : c0 + nb], yos[:, :nb])

    nc.compile()
    return nc


def _get_kernel():
    if "k" not in _KERNEL_CACHE:
        _KERNEL_CACHE["k"] = _build_fused((G + SLOT - 1) // SLOT,
                                          G, NCORES, N, B)
    return _KERNEL_CACHE["k"]


# --------------------------------------------------------------------------
# cached-jit runner (mirrors bass2jax.run_bass_via_pjrt, but caches the
# traced/compiled executable across calls)
# --------------------------------------------------------------------------
def _get_runner(nc, n_cores):
    key = id(nc)
    if key in _RUNNER_CACHE:
        return _RUNNER_CACHE[key]

    import jax
    from jax.experimental.shard_map import shard_map
    from jax.sharding import Mesh, PartitionSpec
    from concourse import bass2jax, mybir

    bass2jax.install_neuronx_cc_hook()
    assert nc.dbg_addr is None
    partition_name = nc.partition_id_tensor.name if nc.partition_id_tensor else None

    in_names, out_names, out_avals, zero_shapes = [], [], [], []
    for alloc in nc.m.functions[0].allocations:
        if not isinstance(alloc, mybir.MemoryLocationSet):
            continue
        name = alloc.memorylocations[0].name
        if alloc.kind == "ExternalInput":
            if name != partition_name:
                in_names.append(name)
        elif alloc.kind == "ExternalOutput":
            out_names.append(name)
            shape = tuple(alloc.tensor_shape)
            dtype = mybir.dt.np(alloc.dtype)
            out_avals.append(jax.core.ShapedArray(shape, dtype))
            zero_shapes.append((shape, dtype))
    n_params = len(in_names)
    all_in_names = (in_names + out_names
                    + ([partition_name] if partition_name else []))
    donate = tuple(range(n_params, n_params + len(out_names)))

    def _body(*args):
        operands = list(args)
        if partition_name is not None:
            operands.append(bass2jax.partition_id_tensor())
        outs = bass2jax._bass_exec_p.bind(
            *operands,
            out_avals=tuple(out_avals),
            in_names=tuple(all_in_names),
            out_names=tuple(out_names),
            lowering_input_output_aliases=(),
            sim_require_finite=True,
            sim_require_nnan=True,
            nc=nc,
        )
        return tuple(outs)

    devices = jax.devices()[:n_cores]
    assert len(devices) == n_cores
    mesh = Mesh(np.asarray(devices), ("core",))
    in_specs = (PartitionSpec("core"),) * (n_params + len(out_names))
    out_specs = (PartitionSpec("core"),) * len(out_names)
    sharded = jax.jit(
        shard_map(_body, mesh=mesh, in_specs=in_specs, out_specs=out_specs,
                  check_rep=False),
        donate_argnums=donate, keep_unused=True,
    )
    runner = (sharded, in_names, out_names, zero_shapes)
    _RUNNER_CACHE[key] = runner
    return runner


def _get_sharding(n_cores):
    key = ("sharding", n_cores)
    if key not in _RUNNER_CACHE:
        import jax
        from jax.sharding import Mesh, PartitionSpec, NamedSharding

        mesh = Mesh(np.asarray(jax.devices()[:n_cores]), ("core",))
        _RUNNER_CACHE[key] = NamedSharding(mesh, PartitionSpec("core"))
    return _RUNNER_CACHE[key]


def _run(nc, in_maps, tag):
    """in_maps: list of per-core dicts (numpy), or a single dict of
    already-concatenated global arrays (numpy or device-resident)."""
    n_cores = NCORES if isinstance(in_maps, dict) else len(in_maps)
    sharded, in_names, out_names, zero_shapes = _get_runner(nc, n_cores)
    if isinstance(in_maps, dict):
        concat_in = [in_maps[name] for name in in_names]
    else:
        concat_in = [
            np.concatenate([np.asarray(in_maps[c][name])
                            for c in range(n_cores)], axis=0)
            for name in in_names
        ]
    concat_zeros = [
        np.zeros((n_cores * shape[0], *shape[1:]), dtype)
        for shape, dtype in zero_shapes
    ]
    out_arrs = sharded(*concat_in, *concat_zeros)
    return [
        {
            name: np.asarray(out_arrs[i]).reshape(
                n_cores, *zero_shapes[i][0])[c]
            for i, name in enumerate(out_names)
        }
        for c in range(n_cores)
    ]


# --------------------------------------------------------------------------
# main entry
# --------------------------------------------------------------------------
def kernel(**inputs) -> np.ndarray:
    x = np.asarray(inputs["x"], np.float32)
    eif = np.asarray(inputs["edge_index_func"])
    eis = np.asarray(inputs["edge_index_struct"])
    ew = np.asarray(inputs["edge_weight_func"], np.float32)
    W1 = np.asarray(inputs["W1"], np.float32)
    g1 = np.asarray(inputs["g1"], np.float32)
    bt1 = np.asarray(inputs["bt1"], np.float32)
    W2 = np.asarray(inputs["W2"], np.float32)
    b2 = np.asarray(inputs["b2"], np.float32)
    g2 = np.asarray(inputs["g2"], np.float32)
    bt2 = np.asarray(inputs["bt2"], np.float32)
    lin1_W = np.asarray(inputs["lin1_W"], np.float32)
    lin1_b = np.asarray(inputs["lin1_b"], np.float32)
    g3 = np.asarray(inputs["g3"], np.float32)
    bt3 = np.asarray(inputs["bt3"], np.float32)
    lin2_W = np.asarray(inputs["lin2_W"], np.float32)
    lin2_b = np.asarray(inputs["lin2_b"], np.float32)
    lin3_W = np.asarray(inputs["lin3_W"], np.float32)
    lin3_b = np.asarray(inputs["lin3_b"], np.float32)

    sf, df = eif[0].astype(np.int64), eif[1].astype(np.int64)
    ss, ds = eis[0].astype(np.int64), eis[1].astype(np.int64)

    # --- structural-assumption checks (else exact numpy fallback) ---
    gs = ss // NN
    ok = np.array_equal(gs, ds // NN) and np.array_equal(
        gs, np.repeat(np.arange(B), ES)
    )
    gf = sf // NN
    ok = ok and np.array_equal(gf, df // NN) and np.array_equal(
        gf, np.repeat(np.arange(B), EF)
    )
    ssl, dsl = ss % NN, ds % NN
    ok = ok and np.array_equal(ssl.reshape(B, ES), np.broadcast_to(ssl[:ES], (B, ES)))
    ok = ok and np.array_equal(dsl.reshape(B, ES), np.broadcast_to(dsl[:ES], (B, ES)))
    ok = ok and np.abs(bt1).max() == 0.0
    if not ok:
        return _fallback_numpy(inputs)

    try:
        return _device_pipeline(x, sf, df, ew, ssl, dsl, W1, g1, W2, b2, g2, bt2,
                                lin1_W, lin1_b, g3, bt3, lin2_W, lin2_b,
                                lin3_W, lin3_b)
    except Exception as e:
        import traceback
        print(f"device pipeline failed ({e}); numpy fallback", file=sys.stderr)
        traceback.print_exc()
        return _fallback_numpy(inputs)


def _pack_block_major(arr_g, ncols, NBLK, NSLOT):
    """[G, NN, ncols] -> [P114, NBLK, ncols] slot-major packing (zero pads)."""
    buf = np.zeros((NSLOT, NN, ncols), np.float32)
    buf[:G] = arr_g
    return np.ascontiguousarray(
        buf.reshape(NBLK, SLOT, NN, ncols).transpose(1, 2, 0, 3)
        .reshape(P114, NBLK, ncols)).astype(BF16)


def _asb_host(ssl, dsl):
    """Shared structural block-diagonal matrix (identical for all graphs)."""
    s0, d0 = ssl[:ES], dsl[:ES]
    deg_s = np.bincount(d0, minlength=NN).astype(np.float64) + 1.0
    dinv_s = 1.0 / np.sqrt(deg_s)
    AsT = np.zeros((NN, NN), np.float64)
    np.add.at(AsT, (s0, d0), dinv_s[s0] * dinv_s[d0])
    AsT[np.arange(NN), np.arange(NN)] += dinv_s * dinv_s
    asb = np.zeros((P114, 128), np.float32)
    for p in range(SLOT):
        asb[p * NN : (p + 1) * NN, p * NN : (p + 1) * NN] = AsT
    return asb.astype(BF16)


def _aft_host(sf, df, ew):
    """Normalized func adjacency, transposed, self-loop folded:
    [B, 19src, 19dst]."""
    deg_f = np.bincount(df, weights=ew.astype(np.float64), minlength=N) + 1.0
    dinv_f = (1.0 / np.sqrt(deg_f)).astype(np.float32)
    norm_f = dinv_f[sf] * ew * dinv_f[df]
    gf = sf // NN
    idx = gf * (NN * NN) + (sf % NN) * NN + (df % NN)
    AfT = np.bincount(idx, weights=norm_f.astype(np.float64),
                      minlength=B * NN * NN).astype(np.float32).reshape(B, NN, NN)
    AfT[:, np.arange(NN), np.arange(NN)] += (dinv_f * dinv_f).reshape(B, NN)
    return AfT


def _small_weights(W1, g1, W2, b2, g2, bt2, lin1_W, lin1_b, g3, bt3,
                   lin2_W, lin2_b, lin3_W, lin3_b):
    w1r = np.ascontiguousarray(W1[:, 0, :])                    # [5, 32]
    # w2k [5, 4, 32]: col (k, sign) -> W2[b, :, k] (same for both signs)
    w2kk = np.empty((BANDS, 4, 32), np.float32)
    for k in range(2):
        for sgn in range(2):
            w2kk[:, k * 2 + sgn, :] = W2[:, :, k]
    cvec = np.ascontiguousarray(
        np.concatenate([b2, g2, bt2], axis=1))                 # [5, 6]
    # lin1 stationary per j=(k,b): w1s[n, j, :] = lin1_W[b*38 + n*2 + k]
    w1s = np.empty((NN, 10, 128), np.float32)
    for k in range(2):
        for b in range(BANDS):
            j = k * BANDS + b
            w1s[:, j, :] = lin1_W[b * 2 * NN + np.arange(NN) * 2 + k]
    l1bv = np.ascontiguousarray(
        np.stack([lin1_b, g3, bt3], axis=1))                   # [128, 3]
    return {
        "w1r": w1r, "g1w": np.ascontiguousarray(g1),
        "w2k": w2kk, "cvec": cvec, "w1s": w1s, "l1bv": l1bv,
        "w2l": np.ascontiguousarray(lin2_W),
        "l2b": lin2_b.reshape(32, 1),
        "w3l": np.ascontiguousarray(lin3_W),
        "l3b": lin3_b.reshape(2, 1),
    }


def _device_pipeline(x, sf, df, ew, ssl, dsl, W1, g1, W2, b2, g2, bt2,
                     lin1_W, lin1_b, g3, bt3, lin2_W, lin2_b, lin3_W, lin3_b):
    import jax

    NBLK = (G + SLOT - 1) // SLOT
    NSLOT = NBLK * SLOT
    nc = _get_kernel()
    sh = _get_sharding(NCORES)
    devs = list(sh.mesh.devices.reshape(-1))

    # big inputs: pack per core, start the (async) transfer immediately so
    # it overlaps the packing of the remaining cores
    AfT = _aft_host(sf, df, ew)
    x3 = x.reshape(B, NN, BANDS)
    afp_shards, xb_shards = [], []
    for c in range(NCORES):
        afp_c = _pack_block_major(AfT[c * G : (c + 1) * G], NN, NBLK, NSLOT)
        afp_shards.append(jax.device_put(afp_c, devs[c]))
        xb_c = _pack_block_major(x3[c * G : (c + 1) * G], BANDS, NBLK, NSLOT)
        xb_shards.append(jax.device_put(xb_c, devs[c]))
    afp_d = jax.make_array_from_single_device_arrays(
        (NCORES * P114, NBLK, NN), sh, afp_shards)
    xb_d = jax.make_array_from_single_device_arrays(
        (NCORES * P114, NBLK, BANDS), sh, xb_shards)

    small = {"asb": _asb_host(ssl, dsl),
             **_small_weights(W1, g1, W2, b2, g2, bt2, lin1_W, lin1_b, g3,
                              bt3, lin2_W, lin2_b, lin3_W, lin3_b)}
    global_map = {"afp": afp_d, "xb": xb_d}
    for name, arr in small.items():
        global_map[name] = np.concatenate([arr] * NCORES, axis=0)

    res = _run(nc, global_map, "fused")

    out = np.empty((B, 2), np.float32)
    for c in range(NCORES):
        out[c * G : (c + 1) * G] = res[c]["yout"][:, :G].T
    return out


# revision 25
# speedup vs baseline: 66.4293x; 1.6743x over previous
"""
Trainium2 Bass kernel for nn_MF_MGCN (5-band 2-layer GCN + MLP head).

Single fused device launch (vs the 4-launch baseline):
  * BatchNorm statistics are reduced across the 8 cores with on-device
    AllReduce collectives (3x tiny: 10, 25, and 256 floats), and the BN
    coefficient algebra runs on-device, so no host round trips remain.
  * The func adjacency ships packed as [114, NBLK, 19] bf16 (~3MB/core
    instead of ~20MB/core block-diagonal); the block-diagonal stationary
    tiles are assembled in SBUF by strided DMAs over a zeroed background.
  * Math identical to the baseline derivation: GCN1 on a 1-channel input
    is rank-1, so relu(BN1) @ W2 collapses onto (relu(z), relu(-z)) and
    GCN2 aggregates just 2 channels/band through one shared structural
    block-diagonal matrix.
  * The executable (jit of shard_map over 8 cores) is cached across
    calls, so warm runs skip retrace/recompile.

All compute-engine operands keep partition base 0 (hardware restricts
bases to {0,32,64}); any partition reshuffling goes through DMA.

If structural assumptions fail (shared struct pattern, bt1 == 0), a pure
numpy fallback reproduces the reference exactly.
"""

import sys

sys.path.insert(0, "/opt/trn_rl_repo")

import numpy as np
import ml_dtypes

BF16 = ml_dtypes.bfloat16

# Problem constants (hardcoded per task contract).
B = 32768
NN = 19
N = B * NN
BANDS = 5
EF, ES = 120, 60
EPS = 1e-5
NCORES = 8
G = B // NCORES           # graphs per core = 4096
SLOT = 6                  # graphs per 114-row block
P114 = SLOT * NN          # 114

_KERNEL_CACHE = {}
_RUNNER_CACHE = {}


# --------------------------------------------------------------------------
# numpy fallback (exact reference math)
# --------------------------------------------------------------------------
def _bn_np(h, g, b):
    m = h.mean(0)
    v = h.var(0)
    return (h - m) / np.sqrt(v + EPS) * g + b


def _gcn_np(h, W, b, src, dst, ew, n):
    h = h @ W
    deg = np.zeros(n, np.float64)
    np.add.at(deg, dst, ew)
    deg += 1.0
    dinv = 1.0 / np.sqrt(deg)
    norm = dinv[src] * ew * dinv[dst]
    agg = np.zeros_like(h, dtype=np.float64)
    np.add.at(agg, dst, norm[:, None] * h[src])
    return agg + (dinv * dinv)[:, None] * h + b


def _fallback_numpy(i):
    x = np.asarray(i["x"], np.float64)
    sf, df = np.asarray(i["edge_index_func"][0]), np.asarray(i["edge_index_func"][1])
    ss, ds = np.asarray(i["edge_index_struct"][0]), np.asarray(i["edge_index_struct"][1])
    ew = np.asarray(i["edge_weight_func"], np.float64)
    ews = np.ones(ss.shape[0], np.float64)
    n = x.shape[0]
    outs = []
    for b in range(BANDS):
        h = _gcn_np(x[:, b : b + 1], np.asarray(i["W1"][b], np.float64),
                    np.asarray(i["b1"][b], np.float64), sf, df, ew, n)
        h = np.maximum(_bn_np(h, np.asarray(i["g1"][b], np.float64),
                              np.asarray(i["bt1"][b], np.float64)), 0)
        h = _gcn_np(h, np.asarray(i["W2"][b], np.float64),
                    np.asarray(i["b2"][b], np.float64), ss, ds, ews, n)
        h = np.maximum(_bn_np(h, np.asarray(i["g2"][b], np.float64),
                              np.asarray(i["bt2"][b], np.float64)), 0)
        outs.append(h.reshape(n // NN, NN * 2))
    xc = np.concatenate(outs, axis=1)
    h = np.maximum(_bn_np(xc @ np.asarray(i["lin1_W"], np.float64)
                          + np.asarray(i["lin1_b"], np.float64),
                          np.asarray(i["g3"], np.float64),
                          np.asarray(i["bt3"], np.float64)), 0)
    h = np.maximum(h @ np.asarray(i["lin2_W"], np.float64)
                   + np.asarray(i["lin2_b"], np.float64), 0)
    out = h @ np.asarray(i["lin3_W"], np.float64) + np.asarray(i["lin3_b"], np.float64)
    return out.astype(np.float32)


# --------------------------------------------------------------------------
# fused Bass kernel builder
# --------------------------------------------------------------------------
def _build_fused(nblk, g_per_core, ncores, n_total, b_total):
    import concourse.bacc as bacc
    import concourse.mybir as mybir
    from concourse import tile

    f32, bf16, u8 = mybir.dt.float32, mybir.dt.bfloat16, mybir.dt.uint8
    Relu = mybir.ActivationFunctionType.Relu
    Sqrt = mybir.ActivationFunctionType.Sqrt
    ALU = mybir.AluOpType
    AX = mybir.AxisListType

    nslot = nblk * SLOT
    npad = nslot - g_per_core          # pad graph slots (live in last block)
    assert 0 <= npad < SLOT
    CH1 = min(48, nblk)                # L1 blocks per chunk
    CH2 = min(51, nblk)                # L2 blocks per chunk (510 moving cols)
    rgroups = [list(range(ncores))]

    nc = bacc.Bacc(None, target_bir_lowering=False, num_devices=ncores)

    xb = nc.dram_tensor("xb", [P114, nblk, BANDS], bf16, kind="ExternalInput")
    afp = nc.dram_tensor("afp", [P114, nblk, NN], u8, kind="ExternalInput")
    asb = nc.dram_tensor("asb", [P114, 128], bf16, kind="ExternalInput")
    w1r = nc.dram_tensor("w1r", [BANDS, 32], f32, kind="ExternalInput")
    g1w = nc.dram_tensor("g1w", [BANDS, 32], f32, kind="ExternalInput")
    w2k = nc.dram_tensor("w2k", [BANDS, 4, 32], f32, kind="ExternalInput")
    cvec = nc.dram_tensor("cvec", [BANDS, 6], f32, kind="ExternalInput")
    w1s = nc.dram_tensor("w1s", [NN, 10, 128], f32, kind="ExternalInput")
    l1bv = nc.dram_tensor("l1bv", [128, 3], f32, kind="ExternalInput")
    w2l = nc.dram_tensor("w2l", [128, 32], f32, kind="ExternalInput")
    l2b = nc.dram_tensor("l2b", [32, 1], f32, kind="ExternalInput")
    w3l = nc.dram_tensor("w3l", [32, 2], f32, kind="ExternalInput")
    l3b = nc.dram_tensor("l3b", [2, 1], f32, kind="ExternalInput")
    yout = nc.dram_tensor("yout", [2, nslot], f32, kind="ExternalOutput")

    inv_n = 1.0 / float(n_total)
    inv_b = 1.0 / float(b_total)

    with tile.TileContext(nc) as tc:
        with (
            tc.tile_pool(name="const", bufs=1) as cp,
            tc.tile_pool(name="big", bufs=1) as bp,
            tc.tile_pool(name="scr", bufs=2) as sp,
            tc.tile_pool(name="scr1", bufs=1) as sp1,
            tc.tile_pool(name="dram", bufs=1, space="DRAM") as dp,
        ):
            # ---------- persistent small loads ----------
            as_t = cp.tile([P114, 128], bf16)
            nc.sync.dma_start(as_t[:], asb[:])
            w1r_t = cp.tile([BANDS, 32], f32)
            nc.sync.dma_start(w1r_t[:], w1r[:])
            g1w_t = cp.tile([BANDS, 32], f32)
            nc.sync.dma_start(g1w_t[:], g1w[:])
            w2k_t = cp.tile([BANDS, 4, 32], f32)
            nc.sync.dma_start(w2k_t[:], w2k[:])
            cv_t = cp.tile([BANDS, 6], f32)
            nc.sync.dma_start(cv_t[:], cvec[:])
            w1s_t = cp.tile([NN, 10, 128], f32)
            nc.sync.dma_start(w1s_t[:], w1s[:])
            l1b_t = cp.tile([128, 3], f32)
            nc.sync.dma_start(l1b_t[:], l1bv[:])
            w2l_t = cp.tile([128, 32], f32)
            nc.sync.dma_start(w2l_t[:], w2l[:])
            l2b_t = cp.tile([32, 1], f32)
            nc.sync.dma_start(l2b_t[:], l2b[:])
            w3l_t = cp.tile([32, 2], f32)
            nc.sync.dma_start(w3l_t[:], w3l[:])
            l3b_t = cp.tile([2, 1], f32)
            nc.sync.dma_start(l3b_t[:], l3b[:])
            ones_t = cp.tile([128, 1], f32)
            nc.vector.memset(ones_t[:], 1.0)
            eps128 = cp.tile([128, 1], f32)
            nc.vector.memset(eps128[:], EPS)
            zpad = cp.tile([P114 - NN, 2 * BANDS], bf16)
            nc.vector.memset(zpad[:], 0.0)

            # DRAM bounce buffers for collectives
            cc1_in = dp.tile([10, 1], f32)
            cc1_out = dp.tile([1, 10], f32)
            cc2_in = dp.tile([25, 1], f32)
            cc2_out = dp.tile([1, 25], f32)
            cc3_in = dp.tile([128, 2], f32)
            cc3_out = dp.tile([128, 2], f32)
            abc_d = dp.tile([1, 30], f32)

            y1_t = bp.tile([128, nblk, SLOT], f32)

            with tc.tile_pool(name="pv", bufs=1) as pv:
                v_t = pv.tile([128, nblk, 2 * BANDS], f32)

                with tc.tile_pool(name="pu", bufs=1) as pu:
                    u_t = pu.tile([P114, nblk, 2 * BANDS], bf16)

                    # ---------- L1: s = AfT_blockdiag @ x ----------
                    with tc.tile_pool(name="p1", bufs=1) as p1:
                        x_t = p1.tile([P114, nblk, BANDS], bf16)
                        nc.sync.dma_start(x_t[:], xb[:])
                        s_t = p1.tile([128, nblk, BANDS], f32)
                        a0 = p1.tile([P114, CH1, 128], bf16)
                        a1 = p1.tile([P114, CH1, 128], bf16)
                        nc.vector.memset(a0[:], 0.0)
                        nc.vector.memset(a1[:], 0.0)
                        a_bufs = [a0, a1]
                        nch1 = (nblk + CH1 - 1) // CH1
                        with tc.tile_pool(name="ps1", bufs=4, space="PSUM") as pp1:
                            for c in range(nch1):
                                c0 = c * CH1
                                nb = min(CH1, nblk - c0)
                                a_t = a_bufs[c % 2]
                                # u8 chunk -> bf16 (per-graph scale is folded
                                # into xb on the host), then scatter into the
                                # block-diagonal positions
                                afq = sp.tile([P114, CH1, NN], u8, tag="afq")
                                nc.sync.dma_start(afq[:, :nb, :],
                                                  afp[:, c0 : c0 + nb, :])
                                afb = sp.tile([P114, CH1, NN], bf16, tag="afb")
                                nc.vector.tensor_copy(out=afb[:, :nb, :],
                                                      in_=afq[:, :nb, :])
                                for p in range(SLOT):
                                    nc.sync.dma_start(
                                        a_t[p * NN : (p + 1) * NN, :nb,
                                            p * NN : (p + 1) * NN],
                                        afb[p * NN : (p + 1) * NN, :nb, :],
                                    )
                                ps = pp1.tile([128, CH1, BANDS], f32, tag="ps1")
                                for j in range(nb):
                                    nc.tensor.matmul(
                                        ps[:, j, :], a_t[:, j, :], x_t[:, c0 + j, :],
                                        start=True, stop=True,
                                    )
                                nc.vector.tensor_copy(out=s_t[:, c0 : c0 + nb, :],
                                                      in_=ps[:, :nb, :])

                            # ---------- L1 stats: sum(s), sum(s^2) per band ----
                            part1 = cp.tile([128, 10], f32)
                            for b in range(BANDS):
                                nc.vector.tensor_reduce(
                                    out=part1[:, b : b + 1], in_=s_t[:, :, b],
                                    axis=AX.X, op=ALU.add)
                                scr = sp.tile([128, nblk], f32, tag="scr")
                                nc.vector.tensor_tensor(
                                    out=scr[:], in0=s_t[:, :, b],
                                    in1=s_t[:, :, b], op=ALU.mult)
                                nc.vector.tensor_reduce(
                                    out=part1[:, 5 + b : 6 + b], in_=scr[:],
                                    axis=AX.X, op=ALU.add)
                            pst1 = pp1.tile([10, 1], f32, tag="pst1")
                            nc.tensor.matmul(pst1[:], part1[:], ones_t[:],
                                             start=True, stop=True)
                            st1_t = cp.tile([10, 1], f32)
                            nc.vector.tensor_copy(out=st1_t[:], in_=pst1[:])

                        nc.sync.dma_start(cc1_in[:], st1_t[:])
                        nc.gpsimd.collective_compute(
                            "AllReduce", ALU.add, replica_groups=rgroups,
                            ins=[cc1_in[:].opt()], outs=[cc1_out[:].opt()],
                        )

                        # ---------- BN1 coefficients ----------
                        mv_t = cp.tile([BANDS, 2], f32)   # col0 mu1, col1 E[s^2]
                        nc.sync.dma_start(
                            mv_t[:, 0:1],
                            cc1_out[:, 0:BANDS].rearrange("one b -> b one"))
                        nc.sync.dma_start(
                            mv_t[:, 1:2],
                            cc1_out[:, BANDS : 2 * BANDS]
                            .rearrange("one b -> b one"))
                        nc.vector.tensor_scalar(out=mv_t[:], in0=mv_t[:],
                                                scalar1=inv_n, scalar2=None,
                                                op0=ALU.mult)
                        var5 = cp.tile([BANDS, 1], f32)
                        nc.vector.tensor_tensor(out=var5[:], in0=mv_t[:, 0:1],
                                                in1=mv_t[:, 0:1], op=ALU.mult)
                        nc.vector.tensor_tensor(out=var5[:], in0=mv_t[:, 1:2],
                                                in1=var5[:], op=ALU.subtract)
                        # a = w1r * rsqrt(var*w1r^2 + eps) * g1   [5, 32]
                        a5 = cp.tile([BANDS, 32], f32)
                        nc.vector.tensor_tensor(out=a5[:], in0=w1r_t[:],
                                                in1=w1r_t[:], op=ALU.mult)
                        nc.vector.tensor_scalar(out=a5[:], in0=a5[:],
                                                scalar1=var5[:, 0:1],
                                                scalar2=None, op0=ALU.mult)
                        nc.scalar.activation(a5[:], a5[:], Sqrt,
                                             bias=eps128[:BANDS, 0:1])
                        nc.vector.reciprocal(a5[:], a5[:])
                        nc.vector.tensor_tensor(out=a5[:], in0=a5[:],
                                                in1=w1r_t[:], op=ALU.mult)
                        nc.vector.tensor_tensor(out=a5[:], in0=a5[:],
                                                in1=g1w_t[:], op=ALU.mult)
                        # apm [5, 4, 32] cols (k, sign)
                        apm = cp.tile([BANDS, 4, 32], f32)
                        nc.vector.tensor_copy(out=apm[:, 0:1, :], in_=a5[:])
                        nc.vector.tensor_scalar(out=apm[:, 1:2, :], in0=a5[:],
                                                scalar1=-1.0, scalar2=None,
                                                op0=ALU.mult)
                        nc.vector.tensor_copy(out=apm[:, 2:3, :],
                                              in_=apm[:, 0:1, :])
                        nc.vector.tensor_copy(out=apm[:, 3:4, :],
                                              in_=apm[:, 1:2, :])
                        nc.scalar.activation(apm[:], apm[:], Relu)
                        # pq [5, 2, 2]: (band, k, sign)
                        prod = cp.tile([BANDS, 4, 32], f32)
                        nc.vector.tensor_tensor(out=prod[:], in0=w2k_t[:],
                                                in1=apm[:], op=ALU.mult)
                        pq = cp.tile([BANDS, 2, 2], f32)
                        nc.vector.tensor_reduce(out=pq[:], in_=prod[:],
                                                axis=AX.X, op=ALU.add)
                        p_ap = pq[:, :, 0:1]
                        q_ap = pq[:, :, 1:2]

                        # mu1 broadcast across partitions for z = s - mu
                        mu_bc = cp.tile([128, BANDS], f32)
                        nc.sync.dma_start(
                            mu_bc[:],
                            cc1_out[:, 0:BANDS].to_broadcast([128, BANDS]))
                        nc.vector.tensor_scalar(out=mu_bc[:], in0=mu_bc[:],
                                                scalar1=inv_n, scalar2=None,
                                                op0=ALU.mult)

                        # ---------- u = relu(+-(s - mu)) ----------
                        nc.vector.tensor_tensor(
                            out=u_t[:, :, 0:BANDS], in0=s_t[:P114],
                            in1=mu_bc[:P114, None, :]
                            .to_broadcast([P114, nblk, BANDS]),
                            op=ALU.subtract,
                        )
                        nc.vector.tensor_scalar(
                            out=u_t[:, :, BANDS : 2 * BANDS],
                            in0=u_t[:, :, 0:BANDS], scalar1=-1.0,
                            scalar2=None, op0=ALU.mult)
                        nc.scalar.activation(u_t[:], u_t[:], Relu)
                        if npad:
                            # zero pad-slot rows via DMA (engine partition
                            # bases are restricted to {0,32,64})
                            nc.sync.dma_start(
                                u_t[(SLOT - npad) * NN : P114, nblk - 1, :],
                                zpad[: npad * NN, :])
                    # p1 closed: x_t / a / s_t freed

                    # ---------- L2: v = As_blockdiag @ u ----------
                    nch2 = (nblk + CH2 - 1) // CH2
                    with tc.tile_pool(name="ps2", bufs=4, space="PSUM") as pp2:
                        for c in range(nch2):
                            c0 = c * CH2
                            nb = min(CH2, nblk - c0)
                            ps = pp2.tile([128, CH2, 2 * BANDS], f32, tag="ps2")
                            nc.tensor.matmul(
                                ps[:, :nb, :], as_t[:], u_t[:, c0 : c0 + nb, :],
                                start=True, stop=True,
                            )
                            nc.vector.tensor_copy(out=v_t[:, c0 : c0 + nb, :],
                                                  in_=ps[:, :nb, :])

                        # ---------- L2 stats ----------
                        part2 = cp.tile([128, 25], f32)
                        for b in range(BANDS):
                            nc.vector.tensor_reduce(
                                out=part2[:, b : b + 1], in_=v_t[:, :, b],
                                axis=AX.X, op=ALU.add)
                            nc.vector.tensor_reduce(
                                out=part2[:, 5 + b : 6 + b],
                                in_=v_t[:, :, 5 + b], axis=AX.X, op=ALU.add)
                            for k, (i0, i1) in ((10, (b, b)),
                                                (15, (5 + b, 5 + b)),
                                                (20, (b, 5 + b))):
                                scr = sp.tile([128, nblk], f32, tag="scr")
                                nc.vector.tensor_tensor(
                                    out=scr[:], in0=v_t[:, :, i0],
                                    in1=v_t[:, :, i1], op=ALU.mult)
                                nc.vector.tensor_reduce(
                                    out=part2[:, k + b : k + b + 1],
                                    in_=scr[:], axis=AX.X, op=ALU.add)
                        pst2 = pp2.tile([25, 1], f32, tag="pst2")
                        nc.tensor.matmul(pst2[:], part2[:], ones_t[:],
                                         start=True, stop=True)
                        st2_t = cp.tile([25, 1], f32)
                        nc.vector.tensor_copy(out=st2_t[:], in_=pst2[:])

                    nc.sync.dma_start(cc2_in[:], st2_t[:])
                    nc.gpsimd.collective_compute(
                        "AllReduce", ALU.add, replica_groups=rgroups,
                        ins=[cc2_in[:].opt()], outs=[cc2_out[:].opt()],
                    )
                # pu closed: u_t freed

                # ---------- BN2 coefficients: A,B,C [5, 2(k)] ----------
                stm5 = cp.tile([BANDS, 5], f32)
                for gidx in range(5):
                    nc.sync.dma_start(
                        stm5[:, gidx : gidx + 1],
                        cc2_out[:, gidx * BANDS : (gidx + 1) * BANDS]
                        .rearrange("one b -> b one"))
                nc.vector.tensor_scalar(out=stm5[:], in0=stm5[:], scalar1=inv_n,
                                        scalar2=None, op0=ALU.mult)
                mom = cp.tile([BANDS, 3], f32)
                nc.vector.tensor_tensor(out=mom[:, 0:1], in0=stm5[:, 0:1],
                                        in1=stm5[:, 0:1], op=ALU.mult)
                nc.vector.tensor_tensor(out=mom[:, 0:1], in0=stm5[:, 2:3],
                                        in1=mom[:, 0:1], op=ALU.subtract)
                nc.vector.tensor_tensor(out=mom[:, 1:2], in0=stm5[:, 1:2],
                                        in1=stm5[:, 1:2], op=ALU.mult)
                nc.vector.tensor_tensor(out=mom[:, 1:2], in0=stm5[:, 3:4],
                                        in1=mom[:, 1:2], op=ALU.subtract)
                nc.vector.tensor_tensor(out=mom[:, 2:3], in0=stm5[:, 0:1],
                                        in1=stm5[:, 1:2], op=ALU.mult)
                nc.vector.tensor_tensor(out=mom[:, 2:3], in0=stm5[:, 4:5],
                                        in1=mom[:, 2:3], op=ALU.subtract)
                mu2 = cp.tile([BANDS, 2], f32)
                t2a = cp.tile([BANDS, 2], f32)
                nc.vector.tensor_scalar(out=mu2[:], in0=p_ap,
                                        scalar1=stm5[:, 0:1],
                                        scalar2=None, op0=ALU.mult)
                nc.vector.tensor_scalar(out=t2a[:], in0=q_ap,
                                        scalar1=stm5[:, 1:2],
                                        scalar2=None, op0=ALU.mult)
                nc.vector.tensor_tensor(out=mu2[:], in0=mu2[:], in1=t2a[:],
                                        op=ALU.add)
                nc.vector.tensor_tensor(out=mu2[:], in0=mu2[:], in1=cv_t[:, 0:2],
                                        op=ALU.add)
                var2 = cp.tile([BANDS, 2], f32)
                nc.vector.tensor_tensor(out=var2[:], in0=p_ap, in1=p_ap,
                                        op=ALU.mult)
                nc.vector.tensor_scalar(out=var2[:], in0=var2[:],
                                        scalar1=mom[:, 0:1],
                                        scalar2=None, op0=ALU.mult)
                nc.vector.tensor_tensor(out=t2a[:], in0=q_ap, in1=q_ap,
                                        op=ALU.mult)
                nc.vector.tensor_scalar(out=t2a[:], in0=t2a[:],
                                        scalar1=mom[:, 1:2],
                                        scalar2=None, op0=ALU.mult)
                nc.vector.tensor_tensor(out=var2[:], in0=var2[:], in1=t2a[:],
                                        op=ALU.add)
                nc.vector.tensor_tensor(out=t2a[:], in0=p_ap, in1=q_ap,
                                        op=ALU.mult)
                nc.vector.tensor_scalar(out=t2a[:], in0=t2a[:],
                                        scalar1=mom[:, 2:3],
                                        scalar2=None, op0=ALU.mult)
                nc.vector.tensor_scalar(out=t2a[:], in0=t2a[:], scalar1=2.0,
                                        scalar2=None, op0=ALU.mult)
                nc.vector.tensor_tensor(out=var2[:], in0=var2[:], in1=t2a[:],
                                        op=ALU.add)
                rs2 = cp.tile([BANDS, 2], f32)
                nc.scalar.activation(rs2[:], var2[:], Sqrt,
                                     bias=eps128[:BANDS, 0:1])
                nc.vector.reciprocal(rs2[:], rs2[:])
                nc.vector.tensor_tensor(out=rs2[:], in0=rs2[:], in1=cv_t[:, 2:4],
                                        op=ALU.mult)   # rsqrt(var+eps)*g2
                abc = cp.tile([BANDS, 6], f32)
                nc.vector.tensor_tensor(out=abc[:, 0:2], in0=p_ap, in1=rs2[:],
                                        op=ALU.mult)
                nc.vector.tensor_tensor(out=abc[:, 2:4], in0=q_ap, in1=rs2[:],
                                        op=ALU.mult)
                nc.vector.tensor_tensor(out=t2a[:], in0=cv_t[:, 0:2], in1=mu2[:],
                                        op=ALU.subtract)
                nc.vector.tensor_tensor(out=t2a[:], in0=t2a[:], in1=rs2[:],
                                        op=ALU.mult)
                nc.vector.tensor_tensor(out=abc[:, 4:6], in0=t2a[:],
                                        in1=cv_t[:, 4:6], op=ALU.add)
                for ci in range(3):
                    nc.sync.dma_start(
                        abc_d[:, ci * 10 : (ci + 1) * 10]
                        .rearrange("one (k b) -> b (one k)", k=2),
                        abc[:, 2 * ci : 2 * ci + 2])
                coef_bc = cp.tile([128, 30], f32)
                nc.sync.dma_start(coef_bc[:], abc_d[:].to_broadcast([128, 30]))

                # ---------- L3: xc = relu(A*v+ + B*v- + C); y1 = lin1(xc) ----
                with (
                    tc.tile_pool(name="p3", bufs=2) as p3,
                    tc.tile_pool(name="ps3", bufs=4, space="PSUM") as pp3,
                ):
                    for s in range(SLOT):
                        vs = p3.tile([NN, nblk, 2 * BANDS], f32, tag="vs")
                        nc.sync.dma_start(vs[:], v_t[s * NN : (s + 1) * NN, :, :])
                        xcs = p3.tile([NN, nblk, 2 * BANDS], f32, tag="xcs")
                        for k in range(2):
                            ksl = slice(k * BANDS, (k + 1) * BANDS)
                            scrb = sp1.tile([NN, nblk, BANDS], f32, tag="scrb")
                            nc.vector.tensor_tensor(
                                out=xcs[:, :, ksl], in0=vs[:, :, 0:BANDS],
                                in1=coef_bc[:NN, None,
                                            k * BANDS : (k + 1) * BANDS]
                                .to_broadcast([NN, nblk, BANDS]),
                                op=ALU.mult,
                            )
                            nc.vector.tensor_tensor(
                                out=scrb[:], in0=vs[:, :, BANDS : 2 * BANDS],
                                in1=coef_bc[:NN, None,
                                            10 + k * BANDS : 10 + (k + 1) * BANDS]
                                .to_broadcast([NN, nblk, BANDS]),
                                op=ALU.mult,
                            )
                            nc.vector.tensor_tensor(out=xcs[:, :, ksl],
                                                    in0=xcs[:, :, ksl],
                                                    in1=scrb[:], op=ALU.add)
                            nc.vector.tensor_tensor(
                                out=xcs[:, :, ksl], in0=xcs[:, :, ksl],
                                in1=coef_bc[:NN, None,
                                            20 + k * BANDS : 20 + (k + 1) * BANDS]
                                .to_broadcast([NN, nblk, BANDS]),
                                op=ALU.add,
                            )
                        nc.scalar.activation(xcs[:], xcs[:], Relu)
                        for c0 in range(0, nblk, 512):
                            nb = min(512, nblk - c0)
                            ps = pp3.tile([128, 512], f32, tag="ps3")
                            for j in range(10):
                                nc.tensor.matmul(
                                    ps[:, :nb], w1s_t[:, j, :],
                                    xcs[:, c0 : c0 + nb, j],
                                    start=(j == 0), stop=(j == 9),
                                )
                            nc.vector.tensor_scalar(
                                out=y1_t[:, c0 : c0 + nb, s], in0=ps[:, :nb],
                                scalar1=l1b_t[:, 0:1], scalar2=None,
                                op0=ALU.add)
            # pv closed: v_t freed
            if npad:
                nc.vector.memset(y1_t[:, nblk - 1, SLOT - npad : SLOT], 0.0)

            # ---------- BN3 stats + head ----------
            with tc.tile_pool(name="ph", bufs=1) as ph:
                part3 = cp.tile([128, 2], f32)
                nc.vector.tensor_reduce(out=part3[:, 0:1], in_=y1_t[:],
                                        axis=AX.XY, op=ALU.add)
                x2_t = ph.tile([128, nblk, SLOT], f32)
                nc.vector.tensor_tensor(out=x2_t[:], in0=y1_t[:], in1=y1_t[:],
                                        op=ALU.mult)
                nc.vector.tensor_reduce(out=part3[:, 1:2], in_=x2_t[:],
                                        axis=AX.XY, op=ALU.add)
                nc.sync.dma_start(cc3_in[:], part3[:])
                nc.gpsimd.collective_compute(
                    "AllReduce", ALU.add, replica_groups=rgroups,
                    ins=[cc3_in[:].opt()], outs=[cc3_out[:].opt()],
                )

                st3_t = cp.tile([128, 2], f32)
                nc.sync.dma_start(st3_t[:], cc3_out[:])
                nc.vector.tensor_scalar(out=st3_t[:], in0=st3_t[:],
                                        scalar1=inv_b, scalar2=None,
                                        op0=ALU.mult)
                g3c = cp.tile([128, 2], f32)        # col0 G3, col1 B3
                nc.vector.tensor_tensor(out=g3c[:, 0:1], in0=st3_t[:, 0:1],
                                        in1=st3_t[:, 0:1], op=ALU.mult)
                nc.vector.tensor_tensor(out=g3c[:, 0:1], in0=st3_t[:, 1:2],
                                        in1=g3c[:, 0:1], op=ALU.subtract)
                nc.scalar.activation(g3c[:, 0:1], g3c[:, 0:1], Sqrt,
                                     bias=eps128[:, 0:1])
                nc.vector.reciprocal(g3c[:, 0:1], g3c[:, 0:1])
                nc.vector.tensor_tensor(out=g3c[:, 0:1], in0=g3c[:, 0:1],
                                        in1=l1b_t[:, 1:2], op=ALU.mult)
                nc.vector.tensor_tensor(out=g3c[:, 1:2], in0=st3_t[:, 0:1],
                                        in1=g3c[:, 0:1], op=ALU.mult)
                nc.vector.tensor_tensor(out=g3c[:, 1:2], in0=l1b_t[:, 2:3],
                                        in1=g3c[:, 1:2], op=ALU.subtract)
                nc.vector.tensor_scalar(out=x2_t[:], in0=y1_t[:],
                                        scalar1=g3c[:, 0:1], scalar2=None,
                                        op0=ALU.mult)
                nc.scalar.activation(x2_t[:], x2_t[:], Relu, bias=g3c[:, 1:2])

                x3_t = ph.tile([32, nslot], f32)
                x2f = x2_t[:].rearrange("p a b -> p (a b)")
                with tc.tile_pool(name="ps4", bufs=4, space="PSUM") as pp4:
                    for c0 in range(0, nslot, 512):
                        nb = min(512, nslot - c0)
                        ps4 = pp4.tile([32, 512], f32, tag="ps4")
                        nc.tensor.matmul(ps4[:, :nb], w2l_t[:],
                                         x2f[:, c0 : c0 + nb],
                                         start=True, stop=True)
                        nc.scalar.activation(x3_t[:, c0 : c0 + nb], ps4[:, :nb],
                                             Relu, bias=l2b_t[:, 0:1])
                    for c0 in range(0, nslot, 512):
                        nb = min(512, nslot - c0)
                        ps5 = pp4.tile([2, 512], f32, tag="ps5")
                        nc.tensor.matmul(ps5[:, :nb], w3l_t[:],
                                         x3_t[:, c0 : c0 + nb],
                                         start=True, stop=True)
                        yos = sp.tile([2, 512], f32, tag="yos")
                        nc.vector.tensor_scalar(out=yos[:, :nb], in0=ps5[:, :nb],
                                                scalar1=l3b_t[:, 0:1],
                                                scalar2=None, op0=ALU.add)
                        nc.sync.dma_start(yout[:, c0 : c0 + nb], yos[:, :nb])

    nc.compile()
    return nc


def _get_kernel():
    if "k" not in _KERNEL_CACHE:
        _KERNEL_CACHE["k"] = _build_fused((G + SLOT - 1) // SLOT,
                                          G, NCORES, N, B)
    return _KERNEL_CACHE["k"]


# --------------------------------------------------------------------------
# cached-jit runner (mirrors bass2jax.run_bass_via_pjrt, but caches the
# traced/compiled executable across calls)
# --------------------------------------------------------------------------
def _get_runner(nc, n_cores):
    key = id(nc)
    if key in _RUNNER_CACHE:
        return _RUNNER_CACHE[key]

    import jax
    from jax.experimental.shard_map import shard_map
    from jax.sharding import Mesh, PartitionSpec
    from concourse import bass2jax, mybir

    bass2jax.install_neuronx_cc_hook()
    assert nc.dbg_addr is None
    partition_name = nc.partition_id_tensor.name if nc.partition_id_tensor else None

    in_names, out_names, out_avals, zero_shapes = [], [], [], []
    for alloc in nc.m.functions[0].allocations:
        if not isinstance(alloc, mybir.MemoryLocationSet):
            continue
        name = alloc.memorylocations[0].name
        if alloc.kind == "ExternalInput":
            if name != partition_name:
                in_names.append(name)
        elif alloc.kind == "ExternalOutput":
            out_names.append(name)
            shape = tuple(alloc.tensor_shape)
            dtype = mybir.dt.np(alloc.dtype)
            out_avals.append(jax.core.ShapedArray(shape, dtype))
            zero_shapes.append((shape, dtype))
    n_params = len(in_names)
    all_in_names = (in_names + out_names
                    + ([partition_name] if partition_name else []))
    donate = tuple(range(n_params, n_params + len(out_names)))

    def _body(*args):
        operands = list(args)
        if partition_name is not None:
            operands.append(bass2jax.partition_id_tensor())
        outs = bass2jax._bass_exec_p.bind(
            *operands,
            out_avals=tuple(out_avals),
            in_names=tuple(all_in_names),
            out_names=tuple(out_names),
            lowering_input_output_aliases=(),
            sim_require_finite=True,
            sim_require_nnan=True,
            nc=nc,
        )
        return tuple(outs)

    devices = jax.devices()[:n_cores]
    assert len(devices) == n_cores
    mesh = Mesh(np.asarray(devices), ("core",))
    in_specs = (PartitionSpec("core"),) * (n_params + len(out_names))
    out_specs = (PartitionSpec("core"),) * len(out_names)
    sharded = jax.jit(
        shard_map(_body, mesh=mesh, in_specs=in_specs, out_specs=out_specs,
                  check_rep=False),
        donate_argnums=donate, keep_unused=True,
    )
    runner = (sharded, in_names, out_names, zero_shapes)
    _RUNNER_CACHE[key] = runner
    return runner


def _get_sharding(n_cores):
    key = ("sharding", n_cores)
    if key not in _RUNNER_CACHE:
        import jax
        from jax.sharding import Mesh, PartitionSpec, NamedSharding

        mesh = Mesh(np.asarray(jax.devices()[:n_cores]), ("core",))
        _RUNNER_CACHE[key] = NamedSharding(mesh, PartitionSpec("core"))
    return _RUNNER_CACHE[key]


def _run(nc, in_maps, tag):
    """in_maps: list of per-core dicts (numpy), or a single dict of
    already-concatenated global arrays (numpy or device-resident)."""
    n_cores = NCORES if isinstance(in_maps, dict) else len(in_maps)
    sharded, in_names, out_names, zero_shapes = _get_runner(nc, n_cores)
    if isinstance(in_maps, dict):
        concat_in = [in_maps[name] for name in in_names]
    else:
        concat_in = [
            np.concatenate([np.asarray(in_maps[c][name])
                            for c in range(n_cores)], axis=0)
            for name in in_names
        ]
    concat_zeros = [
        np.zeros((n_cores * shape[0], *shape[1:]), dtype)
        for shape, dtype in zero_shapes
    ]
    out_arrs = sharded(*concat_in, *concat_zeros)
    return [
        {
            name: np.asarray(out_arrs[i]).reshape(
                n_cores, *zero_shapes[i][0])[c]
            for i, name in enumerate(out_names)
        }
        for c in range(n_cores)
    ]


# --------------------------------------------------------------------------
# main entry
# --------------------------------------------------------------------------
def kernel(**inputs) -> np.ndarray:
    x = np.asarray(inputs["x"], np.float32)
    eif = np.asarray(inputs["edge_index_func"])
    eis = np.asarray(inputs["edge_index_struct"])
    ew = np.asarray(inputs["edge_weight_func"], np.float32)
    W1 = np.asarray(inputs["W1"], np.float32)
    g1 = np.asarray(inputs["g1"], np.float32)
    bt1 = np.asarray(inputs["bt1"], np.float32)
    W2 = np.asarray(inputs["W2"], np.float32)
    b2 = np.asarray(inputs["b2"], np.float32)
    g2 = np.asarray(inputs["g2"], np.float32)
    bt2 = np.asarray(inputs["bt2"], np.float32)
    lin1_W = np.asarray(inputs["lin1_W"], np.float32)
    lin1_b = np.asarray(inputs["lin1_b"], np.float32)
    g3 = np.asarray(inputs["g3"], np.float32)
    bt3 = np.asarray(inputs["bt3"], np.float32)
    lin2_W = np.asarray(inputs["lin2_W"], np.float32)
    lin2_b = np.asarray(inputs["lin2_b"], np.float32)
    lin3_W = np.asarray(inputs["lin3_W"], np.float32)
    lin3_b = np.asarray(inputs["lin3_b"], np.float32)

    sf, df = eif[0].astype(np.int64), eif[1].astype(np.int64)
    ss, ds = eis[0].astype(np.int64), eis[1].astype(np.int64)

    # --- structural-assumption checks (else exact numpy fallback) ---
    gs = ss // NN
    ok = np.array_equal(gs, ds // NN) and np.array_equal(
        gs, np.repeat(np.arange(B), ES)
    )
    gf = sf // NN
    ok = ok and np.array_equal(gf, df // NN) and np.array_equal(
        gf, np.repeat(np.arange(B), EF)
    )
    ssl, dsl = ss % NN, ds % NN
    ok = ok and np.array_equal(ssl.reshape(B, ES), np.broadcast_to(ssl[:ES], (B, ES)))
    ok = ok and np.array_equal(dsl.reshape(B, ES), np.broadcast_to(dsl[:ES], (B, ES)))
    ok = ok and np.abs(bt1).max() == 0.0
    if not ok:
        return _fallback_numpy(inputs)

    try:
        return _device_pipeline(x, sf, df, ew, ssl, dsl, W1, g1, W2, b2, g2, bt2,
                                lin1_W, lin1_b, g3, bt3, lin2_W, lin2_b,
                                lin3_W, lin3_b)
    except Exception as e:
        import traceback
        print(f"device pipeline failed ({e}); numpy fallback", file=sys.stderr)
        traceback.print_exc()
        return _fallback_numpy(inputs)


def _pack_block_major(arr_g, ncols, NBLK, NSLOT, dtype=BF16):
    """[G, NN, ncols] -> [P114, NBLK, ncols] slot-major packing (zero pads)."""
    buf = np.zeros((NSLOT, NN, ncols), arr_g.dtype)
    buf[:G] = arr_g
    return np.ascontiguousarray(
        buf.reshape(NBLK, SLOT, NN, ncols).transpose(1, 2, 0, 3)
        .reshape(P114, NBLK, ncols)).astype(dtype)


def _asb_host(ssl, dsl):
    """Shared structural block-diagonal matrix (identical for all graphs)."""
    s0, d0 = ssl[:ES], dsl[:ES]
    deg_s = np.bincount(d0, minlength=NN).astype(np.float64) + 1.0
    dinv_s = 1.0 / np.sqrt(deg_s)
    AsT = np.zeros((NN, NN), np.float64)
    np.add.at(AsT, (s0, d0), dinv_s[s0] * dinv_s[d0])
    AsT[np.arange(NN), np.arange(NN)] += dinv_s * dinv_s
    asb = np.zeros((P114, 128), np.float32)
    for p in range(SLOT):
        asb[p * NN : (p + 1) * NN, p * NN : (p + 1) * NN] = AsT
    return asb.astype(BF16)


def _aft_host(sf, df, ew):
    """Normalized func adjacency, transposed, self-loop folded:
    [B, 19src, 19dst]."""
    deg_f = np.bincount(df, weights=ew.astype(np.float64), minlength=N) + 1.0
    dinv_f = (1.0 / np.sqrt(deg_f)).astype(np.float32)
    norm_f = dinv_f[sf] * ew * dinv_f[df]
    gf = sf // NN
    idx = gf * (NN * NN) + (sf % NN) * NN + (df % NN)
    AfT = np.bincount(idx, weights=norm_f.astype(np.float64),
                      minlength=B * NN * NN).astype(np.float32).reshape(B, NN, NN)
    AfT[:, np.arange(NN), np.arange(NN)] += (dinv_f * dinv_f).reshape(B, NN)
    return AfT


def _small_weights(W1, g1, W2, b2, g2, bt2, lin1_W, lin1_b, g3, bt3,
                   lin2_W, lin2_b, lin3_W, lin3_b):
    w1r = np.ascontiguousarray(W1[:, 0, :])                    # [5, 32]
    # w2k [5, 4, 32]: col (k, sign) -> W2[b, :, k] (same for both signs)
    w2kk = np.empty((BANDS, 4, 32), np.float32)
    for k in range(2):
        for sgn in range(2):
            w2kk[:, k * 2 + sgn, :] = W2[:, :, k]
    cvec = np.ascontiguousarray(
        np.concatenate([b2, g2, bt2], axis=1))                 # [5, 6]
    # lin1 stationary per j=(k,b): w1s[n, j, :] = lin1_W[b*38 + n*2 + k]
    w1s = np.empty((NN, 10, 128), np.float32)
    for k in range(2):
        for b in range(BANDS):
            j = k * BANDS + b
            w1s[:, j, :] = lin1_W[b * 2 * NN + np.arange(NN) * 2 + k]
    l1bv = np.ascontiguousarray(
        np.stack([lin1_b, g3, bt3], axis=1))                   # [128, 3]
    return {
        "w1r": w1r, "g1w": np.ascontiguousarray(g1),
        "w2k": w2kk, "cvec": cvec, "w1s": w1s, "l1bv": l1bv,
        "w2l": np.ascontiguousarray(lin2_W),
        "l2b": lin2_b.reshape(32, 1),
        "w3l": np.ascontiguousarray(lin3_W),
        "l3b": lin3_b.reshape(2, 1),
    }


def _device_pipeline(x, sf, df, ew, ssl, dsl, W1, g1, W2, b2, g2, bt2,
                     lin1_W, lin1_b, g3, bt3, lin2_W, lin2_b, lin3_W, lin3_b):
    import jax

    NBLK = (G + SLOT - 1) // SLOT
    NSLOT = NBLK * SLOT
    nc = _get_kernel()
    sh = _get_sharding(NCORES)
    devs = list(sh.mesh.devices.reshape(-1))

    # big inputs: pack per core, start the (async) transfer immediately so
    # it overlaps remaining host work. The func adjacency ships as uint8
    # with a per-graph scale folded exactly into xb (s = q @ (x * scale_g)).
    AfT = _aft_host(sf, df, ew)
    if AfT.min() < 0.0:
        # u8 quantization assumes non-negative entries (ew >= 0)
        raise ValueError("negative adjacency entries; using fallback")
    scale_g = (AfT.max(axis=(1, 2)) / 255.0).astype(np.float32)   # > 0
    x3 = x.reshape(B, NN, BANDS) * scale_g[:, None, None]
    xb_shards = []
    for c in range(NCORES):
        xb_c = _pack_block_major(x3[c * G : (c + 1) * G], BANDS, NBLK, NSLOT)
        xb_shards.append(jax.device_put(xb_c, devs[c]))
    afp_shards = []
    for c in range(NCORES):
        q = np.rint(AfT[c * G : (c + 1) * G]
                    / scale_g[c * G : (c + 1) * G, None, None]
                    ).astype(np.uint8)
        afp_c = _pack_block_major(q, NN, NBLK, NSLOT, dtype=np.uint8)
        afp_shards.append(jax.device_put(afp_c, devs[c]))
    afp_d = jax.make_array_from_single_device_arrays(
        (NCORES * P114, NBLK, NN), sh, afp_shards)
    xb_d = jax.make_array_from_single_device_arrays(
        (NCORES * P114, NBLK, BANDS), sh, xb_shards)

    small = {"asb": _asb_host(ssl, dsl),
             **_small_weights(W1, g1, W2, b2, g2, bt2, lin1_W, lin1_b, g3,
                              bt3, lin2_W, lin2_b, lin3_W, lin3_b)}
    global_map = {"afp": afp_d, "xb": xb_d}
    for name, arr in small.items():
        global_map[name] = np.concatenate([arr] * NCORES, axis=0)

    res = _run(nc, global_map, "fused")

    out = np.empty((B, 2), np.float32)
    for c in range(NCORES):
        out[c * G : (c + 1) * G] = res[c]["yout"][:, :G].T
    return out
